# revision 21
# baseline (speedup 1.0000x reference)
"""Trainium2 Bass kernel: Diffusion-DEM PINN total loss (data-parallel, 8 cores).

Strategy
--------
Pure data parallel: every collocation-point set is sharded along axis 0 over
the 8 NeuronCores; the small MLP (2->128->128->128->2, tanh) is replicated.
Derivatives (u,c, ux,cx, ct, uxx,cxx, uxxx) are propagated analytically in
forward mode: each tanh layer carries 5 channels (value, d/dx, d2/dx2,
d3/dx3, d/dt).  The whole channel pipeline runs in fp16 (weights, channels,
chain-rule algebra) -> tensor engine at full rate and vector engine in
2x/4x perf modes; matmul accumulation and the final residual reduction stay
fp32.  Layer-0 input is fed as an exact hi+lo fp16 pair (two accumulating
matmuls) so collocation coordinates keep fp32 accuracy.  Per-point outputs
are staged feature-major to DRAM in fp16 (all out-projections share one
PSUM bank at different partition offsets -> one copy + one DMA per block),
gathered back point-major, and the PDE residuals + squared sums are
computed on-chip in fp32.  Each core returns [128, 10] partial sums; the
host combines them into the scalar loss.

Channel order everywhere: v=0, d/dx=1, d2/dx2=2, d3/dx3=3, d/dt=4.
"""

import os
import numpy as np
from contextlib import ExitStack

import concourse.bass as bass
import concourse.tile as tile
import concourse.mybir as mybir
from concourse import bacc
from concourse.bass_utils import run_bass_kernel_spmd

F32 = mybir.dt.float32
F16 = mybir.dt.bfloat16
AF = mybir.ActivationFunctionType
OP = mybir.AluOpType

M = 8                      # cores
N_INT, N_BND, H = 65536, 8192, 128
NI = N_INT // M            # 8192 interior pts / core
NB = N_BND // M            # 1024 bnd / init pts / core
B = 512                    # point block (PSUM bank width, fp32)
NBI, NBB = NI // B, NB // B
PI, PB = NI // 128, NB // 128   # 64, 8 point-major free dims

# physical constants (from reference)
NU = 0.3
ALFA = 0.001 * 8e-07 * 3.497e-06 / 7.08e-15
THETA = 3.497e-06 * 1e10 / (8.3145 * 300.0 * 3.0 * 0.4)
LAM1 = 1.0 / (1.0 - 2.0 * NU)
LAM2 = (1.0 - NU) / (1.0 + NU)
LAM3 = NU / (1.0 + NU)
K1 = ALFA * 1.3 / (0.7 * 3.0)

OUT_COLS = 10  # fs2, resid2, stress_l2, fd_l2, stress_r2, fd_r2, iu0, iu1, ic0, ic1

# interior staging rows (fp16, [8, NI]): u,ux,uxx,uxxx then c,cx,cxx,ct
IR_U, IR_UX, IR_UXX, IR_UXXX, IR_C, IR_CX, IR_CXX, IR_CT = range(8)
# boundary staging rows (fp16, [5, NB]): u,ux,uxx then c,cx
BR_U, BR_UX, BR_UXX, BR_C, BR_CX = range(5)


def as_ap(v):
    return v if isinstance(v, bass.AP) else v[:]


def emit(nc, tc, ctx, io):
    """Emit the whole per-core program. io maps name -> bass AP (dram)."""
    const = ctx.enter_context(tc.tile_pool(name="const", bufs=1))
    chan = ctx.enter_context(tc.tile_pool(name="chan", bufs=3))
    tmp = ctx.enter_context(tc.tile_pool(name="tmp", bufs=3))
    res = ctx.enter_context(tc.tile_pool(name="res", bufs=1))
    ps_z = ctx.enter_context(tc.tile_pool(name="ps_z", bufs=1, space="PSUM"))
    ps_oA = ctx.enter_context(tc.tile_pool(name="ps_oA", bufs=2, space="PSUM"))
    ps_oB = ctx.enter_context(tc.tile_pool(name="ps_oB", bufs=1, space="PSUM"))
    dram = ctx.enter_context(tc.tile_pool(name="dram", bufs=1, space="DRAM"))

    def load_const(name, shape, dt=F32):
        t = const.tile(list(shape), dt, tag=name, name=name)
        nc.sync.dma_start(t[:], io[name][:])
        return t

    W0h = load_const("W0h", (4, H), F16)
    b0n = load_const("b0n", (H, 1))
    W1 = load_const("W1", (H, H), F16)
    W1x = load_const("W1x", (H, H), F16)
    W1xx = load_const("W1xx", (H, H), F16)
    W1xxx = load_const("W1xxx", (H, H), F16)
    W1t = load_const("W1t", (H, H), F16)
    b1 = load_const("b1", (H, 1))
    W2 = load_const("W2", (H, H), F16)
    b2 = load_const("b2", (H, 1))
    W3s = load_const("W3s", (H, 10), F16)
    b3u = load_const("b3u", (128, 1))
    b3c = load_const("b3c", (128, 1))
    bqh = load_const("bqh", (128, 1))
    XiA_sb = load_const("XiA", (4, NI), F16)
    XlA_sb = load_const("XlA", (4, NB), F16)
    XrA_sb = load_const("XrA", (4, NB), F16)
    X0A_sb = load_const("X0A", (4, NB), F16)
    x_pm = load_const("x_pm", (128, PI))
    xl_pm = load_const("xl_pm", (128, PB))
    xr_pm = load_const("xr_pm", (128, PB))
    Wi0 = load_const("Wi0", (128, 2, PB))
    Yi0 = load_const("Yi0", (128, 2, PB))

    KDBG = int(os.environ.get("KDBG", "0"))

    def ctile(tag):
        return chan.tile([128, B], F16, tag=tag, name=tag)

    def ttile(tag):
        return tmp.tile([128, B], F16, tag=tag, name=tag)

    B2 = 2 * B

    def c2tile(tag):
        return chan.tile([128, B2], F16, tag=tag, name=tag)

    def t2tile(tag):
        return tmp.tile([128, B2], F16, tag=tag, name=tag)

    def layer1(XAB, k0, nch):
        """First layer over a 2-block superblock.  The hi+lo fp16 input pair
        is stacked on the contraction dim (K=4) so one matmul per block gives
        an exactly-rounded fp32 z0.  Returns rhs tiles for the diag-folded
        layer-2 matmuls: [y, p, yp, pq, p]."""
        y = c2tile("h_v")
        for j in range(2):
            z0 = ps_z.tile([128, B], F32, tag="zv", name="z0")
            k = k0 + j
            nc.tensor.matmul(z0[:], lhsT=W0h[:], rhs=XAB[:, k * B : (k + 1) * B])
            nc.scalar.activation(y[:, j * B : (j + 1) * B], z0[:], AF.Tanh,
                                 bias=b0n[:], scale=1.0)
        if nch == 1:
            return [y[:]]
        ysq = t2tile("ysq")
        nc.scalar.activation(ysq[:], y[:], AF.Square)
        p = c2tile("h_p")
        nc.vector.tensor_scalar(p[:], ysq[:], -1.0, 1.0, OP.mult, OP.add)
        yp = c2tile("h_yp")
        nc.vector.tensor_mul(yp[:], y[:], p[:])
        if nch == 3:
            return [y[:], p[:], yp[:]]
        qh = t2tile("qh")
        nc.vector.tensor_scalar(qh[:], ysq[:], 0.5, -1.0 / 6.0, OP.mult, OP.add)
        pq = c2tile("h_pq")
        nc.vector.tensor_mul(pq[:], p[:], qh[:])
        return [y[:], p[:], yp[:], pq[:], p[:]]

    def hidden(chs, Ws, b, nch):
        """One hidden tanh layer over a superblock, rescaled propagation.

        Channel scalings: x: s1=1, xx: s2=-1/2, xxx: s3=1/12, t: 1.  With
        qh = ysq/2 - 1/6 the chain rule is constant-free:
          y1 = p*z1;  y2 = p*(z2 + y*z1^2);  y3 = p*(z3 + z1*(y*z2 + qh*z1^2))
        PSUM stays per 512-block; all SBUF algebra runs at FD=1024.
        """
        Wv, Wx, Wxx, Wxxx, Wt = Ws
        y = c2tile("h_v")
        z1c = t2tile("z1c")
        ztc = t2tile("ztc")
        z2c = t2tile("z2c")
        zxxxs = []
        for j in range(2):
            sl = slice(j * B, (j + 1) * B)
            zv = ps_z.tile([128, B], F32, tag="zv", name="zv")
            nc.tensor.matmul(zv[:], lhsT=Wv[:], rhs=chs[0][:, sl])
            nc.scalar.activation(y[:, sl], zv[:], AF.Tanh, bias=b[:], scale=1.0)
            if nch == 1:
                continue
            zxt = ps_z.tile([128, 2 * B], F32, tag="zxt", name="zxt")
            nc.tensor.matmul(zxt[:, 0:B], lhsT=Wx[:], rhs=chs[1][:, sl])
            zxx = ps_z.tile([128, B], F32, tag="zxx", name="zxx")
            nc.tensor.matmul(zxx[:], lhsT=Wxx[:], rhs=chs[2][:, sl])
            nc.scalar.copy(z1c[:, sl], zxt[:, 0:B])
            if nch == 3:
                nc.vector.tensor_copy(z2c[:, sl], zxx[:])
                continue
            nc.tensor.matmul(zxt[:, B : 2 * B], lhsT=Wt[:], rhs=chs[4][:, sl])
            zxxx = ps_z.tile([128, B], F32, tag="zxxx", name="zxxx")
            nc.tensor.matmul(zxxx[:], lhsT=Wxxx[:], rhs=chs[3][:, sl])
            zxxxs.append(zxxx)
            nc.scalar.copy(ztc[:, sl], zxt[:, B : 2 * B])
            nc.scalar.copy(z2c[:, sl], zxx[:])
        if nch == 1:
            return [y[:]]
        ysq = t2tile("ysq")
        nc.scalar.activation(ysq[:], y[:], AF.Square)
        p = t2tile("p")
        nc.scalar.activation(p[:], ysq[:], AF.Identity, bias=1.0, scale=-1.0)
        z1sq = t2tile("z1sq")
        nc.vector.tensor_mul(z1sq[:], z1c[:], z1c[:])
        y1 = c2tile("h_x")
        nc.vector.tensor_mul(y1[:], p[:], z1c[:])
        g = t2tile("g")
        nc.vector.tensor_mul(g[:], y[:], z1sq[:])
        m2 = t2tile("m2")
        nc.vector.tensor_add(m2[:], g[:], z2c[:])
        y2 = c2tile("h_xx")
        nc.vector.tensor_mul(y2[:], p[:], m2[:])
        if nch == 3:
            return [y[:], y1[:], y2[:]]
        yt = c2tile("h_t")
        nc.vector.tensor_mul(yt[:], p[:], ztc[:])
        qh = t2tile("qh")
        nc.scalar.activation(qh[:], ysq[:], AF.Identity, bias=bqh[:],
                             scale=0.5)
        w1 = t2tile("w1")
        nc.vector.tensor_mul(w1[:], y[:], z2c[:])
        u1 = t2tile("u1")
        nc.vector.tensor_mul(u1[:], qh[:], z1sq[:])
        n3 = t2tile("n3")
        nc.vector.tensor_add(n3[:], w1[:], u1[:])
        n4 = t2tile("n4")
        nc.vector.tensor_mul(n4[:], n3[:], z1c[:])
        n5 = t2tile("n5")
        for j in range(2):
            sl = slice(j * B, (j + 1) * B)
            nc.vector.tensor_add(n5[:, sl], n4[:, sl], zxxxs[j][:])
        y3 = c2tile("h_xxx")
        nc.vector.tensor_mul(y3[:], p[:], n5[:])
        return [y[:], y1[:], y2[:], y3[:], yt[:]]

    def emit_block(XAB, k0, nch, Ud):
        """One 2x512-point superblock of a point set, through the whole net."""
        chs = layer1(XAB, k0, nch)
        W1s = (W1, W1x, W1xx, W1xxx, W1t)
        W2s = (W2, W2, W2, W2, W2)
        chs = hidden(chs, W1s, b1, nch)
        chs = hidden(chs, W2s, b2, nch)
        # out projections at partition bases 0/32/64 of a shared bank; one
        # cast per bank then strided DMAs pick the populated rows.
        nA = min(nch, 3)
        for j in range(2):
            sl = slice(j * B, (j + 1) * B)
            col = slice((k0 + j) * B, (k0 + j + 1) * B)
            ocA = ps_oA.tile([128, B], F32, tag="ocA", name="ocA")
            for c in range(nA):
                nc.tensor.matmul(ocA[32 * c : 32 * c + 2, :],
                                 lhsT=W3s[:, 2 * c : 2 * c + 2],
                                 rhs=chs[c][:, sl])
            if nch == 5:
                ocB = ps_oB.tile([128, B], F32, tag="ocB", name="ocB")
                for jc, c in enumerate((3, 4)):
                    nc.tensor.matmul(ocB[32 * jc : 32 * jc + 2, :],
                                     lhsT=W3s[:, 2 * c : 2 * c + 2],
                                     rhs=chs[c][:, sl])
            if nch == 1:
                ob = tmp.tile([2, B], F16, tag="ob1", name="ob")
                nc.vector.tensor_copy(ob[:], ocA[0:2, :])
                nc.sync.dma_start(Ud[0:2, col], ob[0:2, :])
                continue
            obA = tmp.tile([66, B], F16, tag="obA", name="obA")
            nc.vector.tensor_copy(obA[:], ocA[0:66, :])
            # u-parts: rows u,ux,uxx; c-parts: c,cx(,cxx)
            nc.sync.dma_start(Ud[0:3, col], obA[0:65:32, :])
            if nch == 3:
                nc.sync.dma_start(Ud[3:5, col], obA[1:34:32, :])
                continue
            nc.sync.dma_start(Ud[4:7, col], obA[1:66:32, :])
            obB = tmp.tile([34, B], F16, tag="obB", name="obB")
            nc.vector.tensor_copy(obB[:], ocB[0:34, :])
            nc.sync.dma_start(Ud[3:4, col], obB[0:1, :])
            nc.sync.dma_start(Ud[7:8, col], obB[33:34, :])

    # ---------------- point sets (interleaved schedule) ----------------
    Ud_i = dram.tile([8, NI], F16, tag="Ud_i")
    Ud_l = dram.tile([5, NB], F16, tag="Ud_l")
    Ud_r = dram.tile([5, NB], F16, tag="Ud_r")
    Ud_0 = dram.tile([2, NB], F16, tag="Ud_0")
    bnd_stage = {"l": Ud_l, "r": Ud_r}
    jobs = []
    small = [("l", 0), ("r", 0), ("0", 0)]
    for k in range(NBI // 2):
        jobs.append(("i", 2 * k))
        if k % 3 == 2 and small:
            jobs.append((*small.pop(0),))
    jobs += small
    if KDBG in (2, 3):
        jobs = []
    for set_, k in jobs:
        if set_ == "i":
            emit_block(XiA_sb, k, 5, Ud_i)
        elif set_ == "l":
            emit_block(XlA_sb, k, 3, Ud_l)
        elif set_ == "r":
            emit_block(XrA_sb, k, 3, Ud_r)
        else:
            emit_block(X0A_sb, k, 1, Ud_0)

    # ---------------- output accumulator ----------------
    out_sb = res.tile([128, OUT_COLS], F32, tag="out_sb")
    if KDBG in (1, 3):
        nc.vector.memset(out_sb[:], 0.0)
        nc.sync.dma_start(io["out"][:], out_sb[:])
        return

    def rt(tag, w=PI):
        return res.tile([128, w], F32, tag=tag, name=tag)

    def tt(out, a, b_, op=OP.mult):
        nc.vector.tensor_tensor(as_ap(out), as_ap(a), as_ap(b_), op)
        return out

    def stt(out, in0, scal, in1, op0=OP.mult, op1=OP.add):
        nc.vector.scalar_tensor_tensor(as_ap(out), as_ap(in0), scal,
                                       as_ap(in1), op0, op1)
        return out

    def sumsq(src, colidx, scale=1.0, w=PI):
        dump = res.tile([128, w], F32, tag="dump", name="dump")
        nc.vector.tensor_tensor(dump[:], as_ap(src), as_ap(src), OP.mult)
        nc.vector.tensor_scalar(
            dump[:], dump[:], float(scale), 0.0, OP.mult, OP.add,
            accum_out=out_sb[:, colidx : colidx + 1])

    # ---------------- boundary residuals ----------------
    for si, (setname, xp, rhs) in enumerate((("l", xl_pm, 0.0), ("r", xr_pm, 1.0))):
        Ub = res.tile([128, 5, PB], F16, tag="Ub", name="Ub")
        nc.sync.dma_start(Ub[:], bnd_stage[setname][:].rearrange(
            "c (p i) -> p c i", p=128))
        ub_ = res.tile([128, PB], F32, tag="bu", name="bu")
        nc.vector.tensor_scalar(ub_[:], Ub[:, BR_U, :], b3u[:], None, OP.add)
        cb = res.tile([128, PB], F32, tag="bc", name="bc")
        nc.vector.tensor_scalar(cb[:], Ub[:, BR_C, :], b3c[:], None, OP.add)
        uxb, cxb, uxxb = Ub[:, BR_UX, :], Ub[:, BR_CX, :], Ub[:, BR_UXX, :]
        bx2 = res.tile([128, PB], F32, tag="bx2", name="bx2")
        nc.scalar.activation(bx2[:], xp[:], AF.Square)
        rx = res.tile([128, PB], F32, tag="rx", name="rx")
        nc.vector.reciprocal(rx[:], xp[:])
        t1 = res.tile([128, PB], F32, tag="t1", name="t1")
        tt(t1, ub_, rx)
        t2 = res.tile([128, PB], F32, tag="t2", name="t2")
        nc.vector.tensor_scalar(t2[:], t1[:], LAM3, None, OP.mult)
        t3 = res.tile([128, PB], F32, tag="t3", name="t3")
        nc.vector.scalar_tensor_tensor(t3[:], uxb, LAM2, t2[:], OP.mult, OP.add)
        t4 = res.tile([128, PB], F32, tag="t4", name="t4")
        nc.vector.scalar_tensor_tensor(t4[:], cb[:], -ALFA / 3.0, t3[:],
                                       OP.mult, OP.add)
        sumsq(t4, 2 + 2 * si, scale=LAM1 * LAM1, w=PB)
        m1b = res.tile([128, PB], F32, tag="m1b", name="m1b")
        nc.vector.tensor_tensor(m1b[:], bx2[:], uxxb, OP.mult)
        m2b = res.tile([128, PB], F32, tag="m2b", name="m2b")
        nc.vector.tensor_tensor(m2b[:], xp[:], uxb, OP.mult)
        m3b = res.tile([128, PB], F32, tag="m3b", name="m3b")
        tt(m3b, m1b, m2b, OP.add)
        m4b = res.tile([128, PB], F32, tag="m4b", name="m4b")
        tt(m4b, m3b, ub_, OP.subtract)
        Cb = res.tile([128, PB], F32, tag="Cb", name="Cb")
        nc.vector.tensor_tensor(Cb[:], bx2[:], cxb, OP.mult)
        m5b = res.tile([128, PB], F32, tag="m5b", name="m5b")
        stt(m5b, Cb, -ALFA, m4b)
        m6b = res.tile([128, PB], F32, tag="m6b", name="m6b")
        tt(m6b, cb, m5b)
        fd = res.tile([128, PB], F32, tag="fd", name="fd")
        stt(fd, m6b, -THETA, Cb)
        if rhs != 0.0:
            fd2 = res.tile([128, PB], F32, tag="fd2", name="fd2")
            tt(fd2, fd, bx2, OP.subtract)
            fd = fd2
        sumsq(fd, 3 + 2 * si, w=PB)

    # ---------------- init residuals ----------------
    U0 = res.tile([128, 2, PB], F16, tag="U0")
    nc.sync.dma_start(U0[:], Ud_0[:].rearrange("c (p i) -> p c i", p=128))
    iu = res.tile([128, PB], F32, tag="iu")
    nc.vector.tensor_scalar(iu[:], U0[:, 0, :], b3u[:], None, OP.add)
    ic = res.tile([128, PB], F32, tag="ic")
    nc.vector.tensor_scalar(ic[:], U0[:, 1, :], b3c[:], None, OP.add)
    for oi, val in enumerate((iu, ic)):
        for j in range(2):
            d = res.tile([128, PB], F32, tag="d0", name="d0")
            nc.vector.tensor_tensor(d[:], val[:], Yi0[:, j, :], OP.subtract)
            dw = res.tile([128, PB], F32, tag="dw", name="dw")
            nc.vector.tensor_tensor(dw[:], d[:], Wi0[:, j, :], OP.mult)
            dump = res.tile([128, PB], F32, tag="dump0", name="dump0")
            nc.vector.tensor_tensor(dump[:], d[:], dw[:], OP.mult)
            nc.vector.tensor_scalar(
                dump[:], dump[:], 1.0, 0.0, OP.mult, OP.add,
                accum_out=out_sb[:, 6 + 2 * oi + j : 7 + 2 * oi + j])

    # ---------------- interior residuals ----------------
    Ui = res.tile([128, 8, PI], F16, tag="Ui")
    nc.sync.dma_start(Ui[:], Ud_i[:].rearrange("c (p i) -> p c i", p=128))
    if KDBG == 5:
        nc.vector.memset(out_sb[:], 0.0)
        nc.vector.tensor_copy(out_sb[:, 0:8], Ui[:, :, 0])
        nc.sync.dma_start(io["out"][:], out_sb[:])
        return
    ub_t = rt("ub_t")
    nc.vector.tensor_scalar(ub_t[:], Ui[:, IR_U, :], b3u[:], None, OP.add)
    cb_t = rt("cb_t")
    nc.vector.tensor_scalar(cb_t[:], Ui[:, IR_C, :], b3c[:], None, OP.add)
    x = x_pm
    x2 = rt("x2")
    nc.scalar.activation(x2[:], x[:], AF.Square)
    x3 = tt(rt("x3"), x2, x)
    A = tt(rt("A"), x2[:], Ui[:, IR_UXX, :])          # x2*uxx
    Bt = tt(rt("Bt"), x[:], Ui[:, IR_UX, :])          # x*ux
    C = tt(rt("C"), x2[:], Ui[:, IR_CX, :])           # x2*cx
    D = tt(rt("D"), x3[:], Ui[:, IR_CXX, :])          # x3*cxx
    E = tt(rt("E"), x3[:], Ui[:, IR_UXXX, :])         # x3*uxxx
    F = tt(rt("F"), x3[:], Ui[:, IR_CT, :])           # x3*ct
    j1 = tt(rt("j1"), A, Bt, OP.add)
    j2 = tt(rt("j2"), j1, ub_t, OP.subtract)     # A+B-u
    fs = stt(rt("fs"), C, -K1, j2)               # -K1*C + j2
    sumsq(fs, 0)
    in2 = stt(rt("in2"), C, -ALFA, j2)           # inner2
    i1 = stt(rt("i1"), A, 2.0, Bt, OP.mult, OP.subtract)   # 2A - B
    i2 = tt(rt("i2"), i1, ub_t, OP.add)
    i3 = stt(rt("i3"), C, -ALFA, E)
    i4 = tt(rt("i4"), i2, i3, OP.add)
    i5 = stt(rt("i5"), D, -ALFA, i4)             # inner1
    k1 = tt(rt("k1"), cb_t, i5)
    k2 = tt(rt("k2"), x, Ui[:, IR_CX, :])        # x*cx
    k3 = tt(rt("k3"), k2, in2)
    k4 = stt(rt("k4"), k1, THETA, F)
    k5 = stt(rt("k5"), k3, THETA, k4)
    k7 = tt(rt("k7"), D, C, OP.add)
    r_ = tt(rt("r_"), k5, k7, OP.subtract)
    sumsq(r_, 1)

    nc.sync.dma_start(io["out"][:], out_sb[:])


def build_nc():
    nc = bacc.Bacc("TRN2", target_bir_lowering=False, debug=False, num_devices=M)
    io = {}

    def dp(name, shape, is_out=False, dt=F32):
        h = nc.declare_dram_parameter(name, list(shape), dt, isOutput=is_out)
        io[name] = h.ap()

    dp("XiA", (4, NI), dt=F16)
    dp("XlA", (4, NB), dt=F16)
    dp("XrA", (4, NB), dt=F16)
    dp("X0A", (4, NB), dt=F16)
    dp("x_pm", (128, PI)); dp("xl_pm", (128, PB)); dp("xr_pm", (128, PB))
    dp("W0h", (4, H), dt=F16); dp("b0n", (H, 1))
    dp("W1", (H, H), dt=F16); dp("b1", (H, 1))
    dp("W1x", (H, H), dt=F16); dp("W1xx", (H, H), dt=F16)
    dp("W1xxx", (H, H), dt=F16); dp("W1t", (H, H), dt=F16)
    dp("W2", (H, H), dt=F16); dp("b2", (H, 1))
    dp("W3s", (H, 10), dt=F16); dp("b3u", (128, 1)); dp("b3c", (128, 1))
    dp("bqh", (128, 1))
    dp("Wi0", (128, 2, PB)); dp("Yi0", (128, 2, PB))
    dp("out", (128, OUT_COLS), is_out=True)

    with tile.TileContext(nc) as tc:
        with ExitStack() as ctx:
            emit(nc, tc, ctx, io)
    nc.compile()
    return nc


def host_prep(inputs):
    """Fold normalization into layer-0 weights and build the 8 per-core maps."""
    import ml_dtypes
    f4, f2 = np.float32, ml_dtypes.bfloat16
    g = {k: np.asarray(v) for k, v in inputs.items()}
    Xint = g["Xint"].astype(f4)
    lb = Xint.min(axis=0).astype(np.float64)
    ub = Xint.max(axis=0).astype(np.float64)
    a = 2.0 / (ub - lb)
    W0 = g["W0"].astype(np.float64)
    W0n = a[:, None] * W0
    beta = -2.0 * lb / (ub - lb) - 1.0
    b0n = beta @ W0 + g["b0"].astype(np.float64)
    W0h = W0n.astype(f2)
    w0x = W0h[0].astype(np.float64)   # match device: fp16 weight actually used

    # diag-folded layer-2 weights (channel scalings s1=1, s2=-1/2, s3=1/12
    # are folded here and un-done per-channel in W3s)
    w0t = W0h[1].astype(np.float64)
    W1f = g["W1"].astype(np.float64)
    W3f = g["W3"].astype(np.float64)
    W3sc = np.concatenate([W3f, W3f, -2.0 * W3f, 12.0 * W3f, W3f], axis=1)
    com = {
        "W0h": np.concatenate([W0h, W0h], axis=0),
        "b0n": b0n.astype(f4).reshape(H, 1),
        "W1": g["W1"].astype(f2), "b1": g["b1"].astype(f4).reshape(H, 1),
        "W1x": (w0x[:, None] * W1f).astype(f2),
        "W1xx": (w0x[:, None] ** 2 * W1f).astype(f2),
        "W1xxx": (w0x[:, None] ** 3 * W1f).astype(f2),
        "W1t": (w0t[:, None] * W1f).astype(f2),
        "W2": g["W2"].astype(f2), "b2": g["b2"].astype(f4).reshape(H, 1),
        "W3s": W3sc.astype(f2),
        "b3u": np.full((128, 1), g["b3"][0], f4),
        "b3c": np.full((128, 1), g["b3"][1], f4),
        "bqh": np.full((128, 1), -1.0 / 6.0, f4),
    }

    def split16(X):  # [n,2] fp32 -> stacked hi+lo fp16 [4,n]
        XT = np.ascontiguousarray(X.T.astype(f4))
        A = XT.astype(f2)
        Bx = (XT - A.astype(f4)).astype(f2)
        return np.concatenate([A, Bx], axis=0)

    in_maps = []
    for c in range(M):
        Xi = Xint[c * NI : (c + 1) * NI]
        Xl = g["Xbnd_l"][c * NB : (c + 1) * NB].astype(f4)
        Xr = g["Xbnd_r"][c * NB : (c + 1) * NB].astype(f4)
        X0 = g["Xinit"][c * NB : (c + 1) * NB].astype(f4)
        Wi = g["Winit"][c * NB : (c + 1) * NB].astype(f4)
        Yi = g["Yinit"][c * NB : (c + 1) * NB].astype(f4)
        m = dict(com)
        m["XiA"] = split16(Xi)
        m["XlA"] = split16(Xl)
        m["XrA"] = split16(Xr)
        m["X0A"] = split16(X0)
        m["x_pm"] = np.ascontiguousarray(Xi[:, 0].reshape(128, PI))
        m["xl_pm"] = np.ascontiguousarray(Xl[:, 0].reshape(128, PB))
        m["xr_pm"] = np.ascontiguousarray(Xr[:, 0].reshape(128, PB))
        m["Wi0"] = np.ascontiguousarray(Wi.reshape(128, PB, 2).transpose(0, 2, 1))
        m["Yi0"] = np.ascontiguousarray(Yi.reshape(128, PB, 2).transpose(0, 2, 1))
        in_maps.append(m)
    return in_maps


def combine(results):
    s = np.zeros(OUT_COLS, np.float64)
    for r in results:
        s += r["out"].astype(np.float64).sum(axis=0)
    int_loss = (s[0] + s[1]) / N_INT
    bnd_loss = (s[2] + s[3]) / N_BND + (s[4] + s[5]) / N_BND
    init_loss = (s[6] + s[7] + s[8] + s[9]) / (2 * N_BND)
    return np.float32(int_loss + bnd_loss + init_loss)


_CACHE = {}


def _get_nc():
    if "nc" not in _CACHE:
        _CACHE["nc"] = build_nc()
    return _CACHE["nc"]


def kernel(**inputs):
    in_maps = host_prep(inputs)
    nc = _get_nc()
    res = run_bass_kernel_spmd(nc, in_maps, core_ids=list(range(M)))
    return combine(res.results)


# revision 26
# speedup vs baseline: 1.1982x; 1.1982x over previous
"""Trainium2 Bass kernel: Diffusion-DEM PINN total loss (data-parallel, 8 cores).

Strategy
--------
Pure data parallel: every collocation-point set is sharded along axis 0 over
the 8 NeuronCores; the small MLP (2->128->128->128->2, tanh) is replicated.
Derivatives (u,c, ux,cx, ct, uxx,cxx, uxxx) are propagated analytically in
forward mode: each tanh layer carries 5 channels (value, d/dx, d2/dx2,
d3/dx3, d/dt).  The whole channel pipeline runs in fp16 (weights, channels,
chain-rule algebra) -> tensor engine at full rate and vector engine in
2x/4x perf modes; matmul accumulation and the final residual reduction stay
fp32.  Layer-0 input is fed as an exact hi+lo fp16 pair (two accumulating
matmuls) so collocation coordinates keep fp32 accuracy.  Per-point outputs
are staged feature-major to DRAM in fp16 (all out-projections share one
PSUM bank at different partition offsets -> one copy + one DMA per block),
gathered back point-major, and the PDE residuals + squared sums are
computed on-chip in fp32.  Each core returns [128, 10] partial sums; the
host combines them into the scalar loss.

Channel order everywhere: v=0, d/dx=1, d2/dx2=2, d3/dx3=3, d/dt=4.
"""

import os
import numpy as np
from contextlib import ExitStack

import concourse.bass as bass
import concourse.tile as tile
import concourse.mybir as mybir
from concourse import bacc
from concourse.bass_utils import run_bass_kernel_spmd

F32 = mybir.dt.float32
F16 = mybir.dt.bfloat16
AF = mybir.ActivationFunctionType
OP = mybir.AluOpType

M = 8                      # cores
N_INT, N_BND, H = 65536, 8192, 128
NI = N_INT // M            # 8192 interior pts / core
NB = N_BND // M            # 1024 bnd / init pts / core
B = 512                    # point block (PSUM bank width, fp32)
NBI, NBB = NI // B, NB // B
PI, PB = NI // 128, NB // 128   # 64, 8 point-major free dims

# physical constants (from reference)
NU = 0.3
ALFA = 0.001 * 8e-07 * 3.497e-06 / 7.08e-15
THETA = 3.497e-06 * 1e10 / (8.3145 * 300.0 * 3.0 * 0.4)
LAM1 = 1.0 / (1.0 - 2.0 * NU)
LAM2 = (1.0 - NU) / (1.0 + NU)
LAM3 = NU / (1.0 + NU)
K1 = ALFA * 1.3 / (0.7 * 3.0)

OUT_COLS = 10  # fs2, resid2, stress_l2, fd_l2, stress_r2, fd_r2, iu0, iu1, ic0, ic1

# interior staging rows (fp16, [8, NI]): u,ux,uxx,uxxx then c,cx,cxx,ct
IR_U, IR_UX, IR_UXX, IR_UXXX, IR_C, IR_CX, IR_CXX, IR_CT = range(8)
# boundary staging rows (fp16, [5, NB]): u,ux,uxx then c,cx
BR_U, BR_UX, BR_UXX, BR_C, BR_CX = range(5)


def as_ap(v):
    return v if isinstance(v, bass.AP) else v[:]


def emit(nc, tc, ctx, io):
    """Emit the whole per-core program. io maps name -> bass AP (dram)."""
    const = ctx.enter_context(tc.tile_pool(name="const", bufs=1))
    chan = ctx.enter_context(tc.tile_pool(name="chan", bufs=4))
    tmp = ctx.enter_context(tc.tile_pool(name="tmp", bufs=4))
    res = ctx.enter_context(tc.tile_pool(name="res", bufs=1))
    ps_z = ctx.enter_context(tc.tile_pool(name="ps_z", bufs=1, space="PSUM"))
    ps_oA = ctx.enter_context(tc.tile_pool(name="ps_oA", bufs=2, space="PSUM"))
    ps_oB = ctx.enter_context(tc.tile_pool(name="ps_oB", bufs=1, space="PSUM"))
    dram = ctx.enter_context(tc.tile_pool(name="dram", bufs=1, space="DRAM"))

    def load_const(name, shape, dt=F32):
        t = const.tile(list(shape), dt, tag=name, name=name)
        nc.sync.dma_start(t[:], io[name][:])
        return t

    W0h = load_const("W0h", (2, H), F16)
    b0n = load_const("b0n", (H, 1))
    W1 = load_const("W1", (H, H), F16)
    W1x = load_const("W1x", (H, H), F16)
    W1xx = load_const("W1xx", (H, H), F16)
    W1xxx = load_const("W1xxx", (H, H), F16)
    W1t = load_const("W1t", (H, H), F16)
    b1 = load_const("b1", (H, 1))
    W2 = load_const("W2", (H, H), F16)
    b2 = load_const("b2", (H, 1))
    W3s = load_const("W3s", (H, 10), F16)
    b3u = load_const("b3u", (128, 1))
    b3c = load_const("b3c", (128, 1))
    bqh = load_const("bqh", (128, 1))
    XiA_sb = load_const("XiA", (2, NI), F16)
    XiB_sb = load_const("XiB", (2, NI), F16)
    XlA_sb = load_const("XlA", (2, NB), F16)
    XlB_sb = load_const("XlB", (2, NB), F16)
    XrA_sb = load_const("XrA", (2, NB), F16)
    XrB_sb = load_const("XrB", (2, NB), F16)
    X0A_sb = load_const("X0A", (2, NB), F16)
    X0B_sb = load_const("X0B", (2, NB), F16)
    x_pm = load_const("x_pm", (128, PI))
    xl_pm = load_const("xl_pm", (128, PB))
    xr_pm = load_const("xr_pm", (128, PB))
    Wi0 = load_const("Wi0", (128, 2, PB))
    Yi0 = load_const("Yi0", (128, 2, PB))

    KDBG = int(os.environ.get("KDBG", "0"))

    def ctile(tag):
        return chan.tile([128, B], F16, tag=tag, name=tag)

    def ttile(tag):
        return tmp.tile([128, B], F16, tag=tag, name=tag)

    def layer1(XA, XB, k, nch):
        """First layer from split fp16 coords.  Returns rhs tiles for the
        diag-folded layer-2 matmuls: [y, p, yp, pq, p] (x/t channels both
        use p; xx uses yp; xxx uses pq)."""
        z0 = ps_z.tile([128, B], F32, tag="zv", name="z0")
        nc.tensor.matmul(z0[:], lhsT=W0h[:], rhs=XA[:, k * B : (k + 1) * B],
                         start=True, stop=False)
        nc.tensor.matmul(z0[:], lhsT=W0h[:], rhs=XB[:, k * B : (k + 1) * B],
                         start=False, stop=True)
        y = ctile("h_v")
        nc.scalar.activation(y[:], z0[:], AF.Tanh, bias=b0n[:], scale=1.0)
        if nch == 1:
            return [y[:]]
        ysq = ttile("ysq")
        nc.scalar.activation(ysq[:], y[:], AF.Square)
        p = ctile("h_p")
        nc.vector.tensor_scalar(p[:], ysq[:], -1.0, 1.0, OP.mult, OP.add)
        yp = ctile("h_yp")
        nc.vector.tensor_mul(yp[:], y[:], p[:])
        if nch == 3:
            return [y[:], p[:], yp[:]]
        qh = ttile("qh")
        nc.vector.tensor_scalar(qh[:], ysq[:], 0.5, -1.0 / 6.0, OP.mult, OP.add)
        pq = ctile("h_pq")
        nc.vector.tensor_mul(pq[:], p[:], qh[:])
        return [y[:], p[:], yp[:], pq[:], p[:]]

    def hidden(chs, Ws, b, nch):
        """One hidden tanh layer, rescaled derivative propagation (16-bit).

        Channel scalings: x: s1=1, xx: s2=-1/2, xxx: s3=1/12, t: 1.  With
        qh = ysq/2 - 1/6 the chain rule is constant-free:
          y1 = p*z1;  y2 = p*(z2 + y*z1^2);  y3 = p*(z3 + z1*(y*z2 + qh*z1^2))
        """
        Wv, Wx, Wxx, Wxxx, Wt = Ws
        zv = ps_z.tile([128, B], F32, tag="zv", name="zv")
        nc.tensor.matmul(zv[:], lhsT=Wv[:], rhs=chs[0])
        y = ctile("h_v")
        nc.scalar.activation(y[:], zv[:], AF.Tanh, bias=b[:], scale=1.0)
        if nch == 1:
            return [y[:]]
        zxt = ps_z.tile([128, 2 * B], F32, tag="zxt", name="zxt")
        nc.tensor.matmul(zxt[:, 0:B], lhsT=Wx[:], rhs=chs[1])
        zxx = ps_z.tile([128, B], F32, tag="zxx", name="zxx")
        nc.tensor.matmul(zxx[:], lhsT=Wxx[:], rhs=chs[2])
        if nch == 5:
            nc.tensor.matmul(zxt[:, B : 2 * B], lhsT=Wt[:], rhs=chs[4])
            zxxx = ps_z.tile([128, B], F32, tag="zxxx", name="zxxx")
            nc.tensor.matmul(zxxx[:], lhsT=Wxxx[:], rhs=chs[3])
        z1c = ttile("z1c")
        nc.scalar.copy(z1c[:], zxt[:, 0:B])
        ysq = ttile("ysq")
        nc.scalar.activation(ysq[:], y[:], AF.Square)
        p = ttile("p")
        nc.scalar.activation(p[:], ysq[:], AF.Identity, bias=1.0, scale=-1.0)
        z1sq = ttile("z1sq")
        nc.vector.tensor_mul(z1sq[:], z1c[:], z1c[:])
        y1 = ctile("h_x")
        nc.vector.tensor_mul(y1[:], p[:], z1c[:])
        g = ttile("g")
        nc.vector.tensor_mul(g[:], y[:], z1sq[:])
        if nch == 3:
            m2 = ttile("m2")
            nc.vector.tensor_add(m2[:], g[:], zxx[:])
            y2 = ctile("h_xx")
            nc.vector.tensor_mul(y2[:], p[:], m2[:])
            return [y[:], y1[:], y2[:]]
        z2c = ttile("z2c")
        nc.scalar.copy(z2c[:], zxx[:])
        ztc = ttile("ztc")
        nc.scalar.copy(ztc[:], zxt[:, B : 2 * B])
        yt = ctile("h_t")
        nc.vector.tensor_mul(yt[:], p[:], ztc[:])
        m2 = ttile("m2")
        nc.vector.tensor_add(m2[:], g[:], z2c[:])
        y2 = ctile("h_xx")
        nc.vector.tensor_mul(y2[:], p[:], m2[:])
        qh = ttile("qh")
        nc.scalar.activation(qh[:], ysq[:], AF.Identity, bias=bqh[:],
                             scale=0.5)
        w1 = ttile("w1")
        nc.vector.tensor_mul(w1[:], y[:], z2c[:])
        u1 = ttile("u1")
        nc.vector.tensor_mul(u1[:], qh[:], z1sq[:])
        n3 = ttile("n3")
        nc.vector.tensor_add(n3[:], w1[:], u1[:])
        n4 = ttile("n4")
        nc.vector.tensor_mul(n4[:], n3[:], z1c[:])
        n5 = ttile("n5")
        nc.vector.tensor_add(n5[:], n4[:], zxxx[:])
        y3 = ctile("h_xxx")
        nc.vector.tensor_mul(y3[:], p[:], n5[:])
        return [y[:], y1[:], y2[:], y3[:], yt[:]]

    def emit_block(XA, XB, k, nch, Ud):
        """One 512-point block of a point set, through the whole net."""
        chs = layer1(XA, XB, k, nch)
        W1s = (W1, W1x, W1xx, W1xxx, W1t)
        W2s = (W2, W2, W2, W2, W2)
        chs = hidden(chs, W1s, b1, nch)
        chs = hidden(chs, W2s, b2, nch)
        # out projections at partition bases 0/32/64 of a shared bank; one
        # cast per bank then strided DMAs pick the populated rows.
        nA = min(nch, 3)
        ocA = ps_oA.tile([128, B], F32, tag="ocA", name="ocA")
        for c in range(nA):
            nc.tensor.matmul(ocA[32 * c : 32 * c + 2, :],
                             lhsT=W3s[:, 2 * c : 2 * c + 2], rhs=chs[c])
        if nch == 5:
            ocB = ps_oB.tile([128, B], F32, tag="ocB", name="ocB")
            for j, c in enumerate((3, 4)):
                nc.tensor.matmul(ocB[32 * j : 32 * j + 2, :],
                                 lhsT=W3s[:, 2 * c : 2 * c + 2], rhs=chs[c])
        col = slice(k * B, (k + 1) * B)
        if nch == 1:
            ob = tmp.tile([2, B], F16, tag="ob1", name="ob")
            nc.vector.tensor_copy(ob[:], ocA[0:2, :])
            nc.sync.dma_start(Ud[0:2, col], ob[0:2, :])
            return
        obA = tmp.tile([66, B], F16, tag="obA", name="obA")
        if k % 2 == 0:
            nc.vector.tensor_copy(obA[:], ocA[0:66, :])
        else:
            nc.scalar.copy(obA[:], ocA[0:66, :])
        # u-parts: rows u,ux,uxx; c-parts: c,cx(,cxx)
        nc.sync.dma_start(Ud[0:3, col], obA[0:65:32, :])
        if nch == 3:
            nc.sync.dma_start(Ud[3:5, col], obA[1:34:32, :])
            return
        nc.sync.dma_start(Ud[4:7, col], obA[1:66:32, :])
        obB = tmp.tile([34, B], F16, tag="obB", name="obB")
        nc.vector.tensor_copy(obB[:], ocB[0:34, :])
        nc.sync.dma_start(Ud[3:4, col], obB[0:1, :])
        nc.sync.dma_start(Ud[7:8, col], obB[33:34, :])

    # ---------------- point sets (interleaved schedule) ----------------
    Ud_i = dram.tile([8, NI], F16, tag="Ud_i")
    Ud_l = dram.tile([5, NB], F16, tag="Ud_l")
    Ud_r = dram.tile([5, NB], F16, tag="Ud_r")
    Ud_0 = dram.tile([2, NB], F16, tag="Ud_0")
    bnd_stage = {"l": Ud_l, "r": Ud_r}
    jobs = []
    small = [("l", 0), ("l", 1), ("r", 0), ("r", 1), ("0", 0), ("0", 1)]
    for k in range(NBI):
        jobs.append(("i", k))
        if k % 3 == 2 and small:
            jobs.append(small.pop(0))
    jobs += small
    if KDBG in (2, 3):
        jobs = []
    for set_, k in jobs:
        if set_ == "i":
            emit_block(XiA_sb, XiB_sb, k, 5, Ud_i)
        elif set_ == "l":
            emit_block(XlA_sb, XlB_sb, k, 3, Ud_l)
        elif set_ == "r":
            emit_block(XrA_sb, XrB_sb, k, 3, Ud_r)
        else:
            emit_block(X0A_sb, X0B_sb, k, 1, Ud_0)

    # ---------------- output accumulator ----------------
    out_sb = res.tile([128, OUT_COLS], F32, tag="out_sb")
    if KDBG in (1, 3):
        nc.vector.memset(out_sb[:], 0.0)
        nc.sync.dma_start(io["out"][:], out_sb[:])
        return

    def rt(tag, w=PI):
        return res.tile([128, w], F32, tag=tag, name=tag)

    def tt(out, a, b_, op=OP.mult):
        nc.vector.tensor_tensor(as_ap(out), as_ap(a), as_ap(b_), op)
        return out

    def stt(out, in0, scal, in1, op0=OP.mult, op1=OP.add):
        nc.vector.scalar_tensor_tensor(as_ap(out), as_ap(in0), scal,
                                       as_ap(in1), op0, op1)
        return out

    def sumsq(src, colidx, scale=1.0, w=PI):
        dump = res.tile([128, w], F32, tag="dump", name="dump")
        nc.vector.tensor_tensor(dump[:], as_ap(src), as_ap(src), OP.mult)
        nc.vector.tensor_scalar(
            dump[:], dump[:], float(scale), 0.0, OP.mult, OP.add,
            accum_out=out_sb[:, colidx : colidx + 1])

    # ---------------- boundary residuals ----------------
    for si, (setname, xp, rhs) in enumerate((("l", xl_pm, 0.0), ("r", xr_pm, 1.0))):
        Ub = res.tile([128, 5, PB], F16, tag="Ub", name="Ub")
        nc.sync.dma_start(Ub[:], bnd_stage[setname][:].rearrange(
            "c (p i) -> p c i", p=128))
        ub_ = res.tile([128, PB], F32, tag="bu", name="bu")
        nc.vector.tensor_scalar(ub_[:], Ub[:, BR_U, :], b3u[:], None, OP.add)
        cb = res.tile([128, PB], F32, tag="bc", name="bc")
        nc.vector.tensor_scalar(cb[:], Ub[:, BR_C, :], b3c[:], None, OP.add)
        uxb, cxb, uxxb = Ub[:, BR_UX, :], Ub[:, BR_CX, :], Ub[:, BR_UXX, :]
        bx2 = res.tile([128, PB], F32, tag="bx2", name="bx2")
        nc.scalar.activation(bx2[:], xp[:], AF.Square)
        rx = res.tile([128, PB], F32, tag="rx", name="rx")
        nc.vector.reciprocal(rx[:], xp[:])
        t1 = res.tile([128, PB], F32, tag="t1", name="t1")
        tt(t1, ub_, rx)
        t2 = res.tile([128, PB], F32, tag="t2", name="t2")
        nc.vector.tensor_scalar(t2[:], t1[:], LAM3, None, OP.mult)
        t3 = res.tile([128, PB], F32, tag="t3", name="t3")
        nc.vector.scalar_tensor_tensor(t3[:], uxb, LAM2, t2[:], OP.mult, OP.add)
        t4 = res.tile([128, PB], F32, tag="t4", name="t4")
        nc.vector.scalar_tensor_tensor(t4[:], cb[:], -ALFA / 3.0, t3[:],
                                       OP.mult, OP.add)
        sumsq(t4, 2 + 2 * si, scale=LAM1 * LAM1, w=PB)
        m1b = res.tile([128, PB], F32, tag="m1b", name="m1b")
        nc.vector.tensor_tensor(m1b[:], bx2[:], uxxb, OP.mult)
        m2b = res.tile([128, PB], F32, tag="m2b", name="m2b")
        nc.vector.tensor_tensor(m2b[:], xp[:], uxb, OP.mult)
        m3b = res.tile([128, PB], F32, tag="m3b", name="m3b")
        tt(m3b, m1b, m2b, OP.add)
        m4b = res.tile([128, PB], F32, tag="m4b", name="m4b")
        tt(m4b, m3b, ub_, OP.subtract)
        Cb = res.tile([128, PB], F32, tag="Cb", name="Cb")
        nc.vector.tensor_tensor(Cb[:], bx2[:], cxb, OP.mult)
        m5b = res.tile([128, PB], F32, tag="m5b", name="m5b")
        stt(m5b, Cb, -ALFA, m4b)
        m6b = res.tile([128, PB], F32, tag="m6b", name="m6b")
        tt(m6b, cb, m5b)
        fd = res.tile([128, PB], F32, tag="fd", name="fd")
        stt(fd, m6b, -THETA, Cb)
        if rhs != 0.0:
            fd2 = res.tile([128, PB], F32, tag="fd2", name="fd2")
            tt(fd2, fd, bx2, OP.subtract)
            fd = fd2
        sumsq(fd, 3 + 2 * si, w=PB)

    # ---------------- init residuals ----------------
    U0 = res.tile([128, 2, PB], F16, tag="U0")
    nc.sync.dma_start(U0[:], Ud_0[:].rearrange("c (p i) -> p c i", p=128))
    iu = res.tile([128, PB], F32, tag="iu")
    nc.vector.tensor_scalar(iu[:], U0[:, 0, :], b3u[:], None, OP.add)
    ic = res.tile([128, PB], F32, tag="ic")
    nc.vector.tensor_scalar(ic[:], U0[:, 1, :], b3c[:], None, OP.add)
    for oi, val in enumerate((iu, ic)):
        for j in range(2):
            d = res.tile([128, PB], F32, tag="d0", name="d0")
            nc.vector.tensor_tensor(d[:], val[:], Yi0[:, j, :], OP.subtract)
            dw = res.tile([128, PB], F32, tag="dw", name="dw")
            nc.vector.tensor_tensor(dw[:], d[:], Wi0[:, j, :], OP.mult)
            dump = res.tile([128, PB], F32, tag="dump0", name="dump0")
            nc.vector.tensor_tensor(dump[:], d[:], dw[:], OP.mult)
            nc.vector.tensor_scalar(
                dump[:], dump[:], 1.0, 0.0, OP.mult, OP.add,
                accum_out=out_sb[:, 6 + 2 * oi + j : 7 + 2 * oi + j])

    # ---------------- interior residuals ----------------
    Ui = res.tile([128, 8, PI], F16, tag="Ui")
    nc.sync.dma_start(Ui[:], Ud_i[:].rearrange("c (p i) -> p c i", p=128))
    if KDBG == 5:
        nc.vector.memset(out_sb[:], 0.0)
        nc.vector.tensor_copy(out_sb[:, 0:8], Ui[:, :, 0])
        nc.sync.dma_start(io["out"][:], out_sb[:])
        return
    ub_t = rt("ub_t")
    nc.vector.tensor_scalar(ub_t[:], Ui[:, IR_U, :], b3u[:], None, OP.add)
    cb_t = rt("cb_t")
    nc.vector.tensor_scalar(cb_t[:], Ui[:, IR_C, :], b3c[:], None, OP.add)
    x = x_pm
    x2 = rt("x2")
    nc.scalar.activation(x2[:], x[:], AF.Square)
    x3 = tt(rt("x3"), x2, x)
    A = tt(rt("A"), x2[:], Ui[:, IR_UXX, :])          # x2*uxx
    Bt = tt(rt("Bt"), x[:], Ui[:, IR_UX, :])          # x*ux
    C = tt(rt("C"), x2[:], Ui[:, IR_CX, :])           # x2*cx
    D = tt(rt("D"), x3[:], Ui[:, IR_CXX, :])          # x3*cxx
    E = tt(rt("E"), x3[:], Ui[:, IR_UXXX, :])         # x3*uxxx
    F = tt(rt("F"), x3[:], Ui[:, IR_CT, :])           # x3*ct
    j1 = tt(rt("j1"), A, Bt, OP.add)
    j2 = tt(rt("j2"), j1, ub_t, OP.subtract)     # A+B-u
    fs = stt(rt("fs"), C, -K1, j2)               # -K1*C + j2
    sumsq(fs, 0)
    in2 = stt(rt("in2"), C, -ALFA, j2)           # inner2
    i1 = stt(rt("i1"), A, 2.0, Bt, OP.mult, OP.subtract)   # 2A - B
    i2 = tt(rt("i2"), i1, ub_t, OP.add)
    i3 = stt(rt("i3"), C, -ALFA, E)
    i4 = tt(rt("i4"), i2, i3, OP.add)
    i5 = stt(rt("i5"), D, -ALFA, i4)             # inner1
    k1 = tt(rt("k1"), cb_t, i5)
    k2 = tt(rt("k2"), x, Ui[:, IR_CX, :])        # x*cx
    k3 = tt(rt("k3"), k2, in2)
    k4 = stt(rt("k4"), k1, THETA, F)
    k5 = stt(rt("k5"), k3, THETA, k4)
    k7 = tt(rt("k7"), D, C, OP.add)
    r_ = tt(rt("r_"), k5, k7, OP.subtract)
    sumsq(r_, 1)

    nc.sync.dma_start(io["out"][:], out_sb[:])


def build_nc():
    nc = bacc.Bacc("TRN2", target_bir_lowering=False, debug=False, num_devices=M)
    io = {}

    def dp(name, shape, is_out=False, dt=F32):
        h = nc.declare_dram_parameter(name, list(shape), dt, isOutput=is_out)
        io[name] = h.ap()

    dp("XiA", (2, NI), dt=F16); dp("XiB", (2, NI), dt=F16)
    dp("XlA", (2, NB), dt=F16); dp("XlB", (2, NB), dt=F16)
    dp("XrA", (2, NB), dt=F16); dp("XrB", (2, NB), dt=F16)
    dp("X0A", (2, NB), dt=F16); dp("X0B", (2, NB), dt=F16)
    dp("x_pm", (128, PI)); dp("xl_pm", (128, PB)); dp("xr_pm", (128, PB))
    dp("W0h", (2, H), dt=F16); dp("b0n", (H, 1))
    dp("W1", (H, H), dt=F16); dp("b1", (H, 1))
    dp("W1x", (H, H), dt=F16); dp("W1xx", (H, H), dt=F16)
    dp("W1xxx", (H, H), dt=F16); dp("W1t", (H, H), dt=F16)
    dp("W2", (H, H), dt=F16); dp("b2", (H, 1))
    dp("W3s", (H, 10), dt=F16); dp("b3u", (128, 1)); dp("b3c", (128, 1))
    dp("bqh", (128, 1))
    dp("Wi0", (128, 2, PB)); dp("Yi0", (128, 2, PB))
    dp("out", (128, OUT_COLS), is_out=True)

    with tile.TileContext(nc) as tc:
        with ExitStack() as ctx:
            emit(nc, tc, ctx, io)
    nc.compile()
    return nc


def host_prep(inputs):
    """Fold normalization into layer-0 weights and build the 8 per-core maps."""
    import ml_dtypes
    f4, f2 = np.float32, ml_dtypes.bfloat16
    g = {k: np.asarray(v) for k, v in inputs.items()}
    Xint = g["Xint"].astype(f4)
    lb = Xint.min(axis=0).astype(np.float64)
    ub = Xint.max(axis=0).astype(np.float64)
    a = 2.0 / (ub - lb)
    W0 = g["W0"].astype(np.float64)
    W0n = a[:, None] * W0
    beta = -2.0 * lb / (ub - lb) - 1.0
    b0n = beta @ W0 + g["b0"].astype(np.float64)
    W0h = W0n.astype(f2)
    w0x = W0h[0].astype(np.float64)   # match device: fp16 weight actually used

    # diag-folded layer-2 weights (channel scalings s1=1, s2=-1/2, s3=1/12
    # are folded here and un-done per-channel in W3s)
    w0t = W0h[1].astype(np.float64)
    W1f = g["W1"].astype(np.float64)
    W3f = g["W3"].astype(np.float64)
    W3sc = np.concatenate([W3f, W3f, -2.0 * W3f, 12.0 * W3f, W3f], axis=1)
    com = {
        "W0h": W0h,
        "b0n": b0n.astype(f4).reshape(H, 1),
        "W1": g["W1"].astype(f2), "b1": g["b1"].astype(f4).reshape(H, 1),
        "W1x": (w0x[:, None] * W1f).astype(f2),
        "W1xx": (w0x[:, None] ** 2 * W1f).astype(f2),
        "W1xxx": (w0x[:, None] ** 3 * W1f).astype(f2),
        "W1t": (w0t[:, None] * W1f).astype(f2),
        "W2": g["W2"].astype(f2), "b2": g["b2"].astype(f4).reshape(H, 1),
        "W3s": W3sc.astype(f2),
        "b3u": np.full((128, 1), g["b3"][0], f4),
        "b3c": np.full((128, 1), g["b3"][1], f4),
        "bqh": np.full((128, 1), -1.0 / 6.0, f4),
    }

    def split16(X):  # [n,2] fp32 -> (hi, lo) fp16 transposed [2,n]
        XT = np.ascontiguousarray(X.T.astype(f4))
        A = XT.astype(f2)
        Bx = (XT - A.astype(f4)).astype(f2)
        return A, Bx

    in_maps = []
    for c in range(M):
        Xi = Xint[c * NI : (c + 1) * NI]
        Xl = g["Xbnd_l"][c * NB : (c + 1) * NB].astype(f4)
        Xr = g["Xbnd_r"][c * NB : (c + 1) * NB].astype(f4)
        X0 = g["Xinit"][c * NB : (c + 1) * NB].astype(f4)
        Wi = g["Winit"][c * NB : (c + 1) * NB].astype(f4)
        Yi = g["Yinit"][c * NB : (c + 1) * NB].astype(f4)
        m = dict(com)
        m["XiA"], m["XiB"] = split16(Xi)
        m["XlA"], m["XlB"] = split16(Xl)
        m["XrA"], m["XrB"] = split16(Xr)
        m["X0A"], m["X0B"] = split16(X0)
        m["x_pm"] = np.ascontiguousarray(Xi[:, 0].reshape(128, PI))
        m["xl_pm"] = np.ascontiguousarray(Xl[:, 0].reshape(128, PB))
        m["xr_pm"] = np.ascontiguousarray(Xr[:, 0].reshape(128, PB))
        m["Wi0"] = np.ascontiguousarray(Wi.reshape(128, PB, 2).transpose(0, 2, 1))
        m["Yi0"] = np.ascontiguousarray(Yi.reshape(128, PB, 2).transpose(0, 2, 1))
        in_maps.append(m)
    return in_maps


def combine(results):
    s = np.zeros(OUT_COLS, np.float64)
    for r in results:
        s += r["out"].astype(np.float64).sum(axis=0)
    int_loss = (s[0] + s[1]) / N_INT
    bnd_loss = (s[2] + s[3]) / N_BND + (s[4] + s[5]) / N_BND
    init_loss = (s[6] + s[7] + s[8] + s[9]) / (2 * N_BND)
    return np.float32(int_loss + bnd_loss + init_loss)


_CACHE = {}


def _get_nc():
    if "nc" not in _CACHE:
        _CACHE["nc"] = build_nc()
    return _CACHE["nc"]


def kernel(**inputs):
    in_maps = host_prep(inputs)
    nc = _get_nc()
    res = run_bass_kernel_spmd(nc, in_maps, core_ids=list(range(M)))
    return combine(res.results)


# revision 27
# speedup vs baseline: 1.2025x; 1.0036x over previous
"""Trainium2 Bass kernel: Diffusion-DEM PINN total loss (data-parallel, 8 cores).

Strategy
--------
Pure data parallel: every collocation-point set is sharded along axis 0 over
the 8 NeuronCores; the small MLP (2->128->128->128->2, tanh) is replicated.
Derivatives (u,c, ux,cx, ct, uxx,cxx, uxxx) are propagated analytically in
forward mode: each tanh layer carries 5 channels (value, d/dx, d2/dx2,
d3/dx3, d/dt).  The whole channel pipeline runs in fp16 (weights, channels,
chain-rule algebra) -> tensor engine at full rate and vector engine in
2x/4x perf modes; matmul accumulation and the final residual reduction stay
fp32.  Layer-0 input is fed as an exact hi+lo fp16 pair (two accumulating
matmuls) so collocation coordinates keep fp32 accuracy.  Per-point outputs
are staged feature-major to DRAM in fp16 (all out-projections share one
PSUM bank at different partition offsets -> one copy + one DMA per block),
gathered back point-major, and the PDE residuals + squared sums are
computed on-chip in fp32.  Each core returns [128, 10] partial sums; the
host combines them into the scalar loss.

Channel order everywhere: v=0, d/dx=1, d2/dx2=2, d3/dx3=3, d/dt=4.
"""

import os
import numpy as np
from contextlib import ExitStack

import concourse.bass as bass
import concourse.tile as tile
import concourse.mybir as mybir
from concourse import bacc
from concourse.bass_utils import run_bass_kernel_spmd

F32 = mybir.dt.float32
F16 = mybir.dt.bfloat16
AF = mybir.ActivationFunctionType
OP = mybir.AluOpType

M = 8                      # cores
N_INT, N_BND, H = 65536, 8192, 128
NI = N_INT // M            # 8192 interior pts / core
NB = N_BND // M            # 1024 bnd / init pts / core
B = 512                    # point block (PSUM bank width, fp32)
NBI, NBB = NI // B, NB // B
PI, PB = NI // 128, NB // 128   # 64, 8 point-major free dims

# physical constants (from reference)
NU = 0.3
ALFA = 0.001 * 8e-07 * 3.497e-06 / 7.08e-15
THETA = 3.497e-06 * 1e10 / (8.3145 * 300.0 * 3.0 * 0.4)
LAM1 = 1.0 / (1.0 - 2.0 * NU)
LAM2 = (1.0 - NU) / (1.0 + NU)
LAM3 = NU / (1.0 + NU)
K1 = ALFA * 1.3 / (0.7 * 3.0)

OUT_COLS = 10  # fs2, resid2, stress_l2, fd_l2, stress_r2, fd_r2, iu0, iu1, ic0, ic1

# interior staging rows (fp16, [8, NI]): u,ux,uxx,uxxx then c,cx,cxx,ct
IR_U, IR_UX, IR_UXX, IR_UXXX, IR_C, IR_CX, IR_CXX, IR_CT = range(8)
# boundary staging rows (fp16, [5, NB]): u,ux,uxx then c,cx
BR_U, BR_UX, BR_UXX, BR_C, BR_CX = range(5)


def as_ap(v):
    return v if isinstance(v, bass.AP) else v[:]


def emit(nc, tc, ctx, io):
    """Emit the whole per-core program. io maps name -> bass AP (dram)."""
    const = ctx.enter_context(tc.tile_pool(name="const", bufs=1))
    chan = ctx.enter_context(tc.tile_pool(name="chan", bufs=5))
    tmp = ctx.enter_context(tc.tile_pool(name="tmp", bufs=5))
    res = ctx.enter_context(tc.tile_pool(name="res", bufs=1))
    ps_z = ctx.enter_context(tc.tile_pool(name="ps_z", bufs=1, space="PSUM"))
    ps_oA = ctx.enter_context(tc.tile_pool(name="ps_oA", bufs=2, space="PSUM"))
    ps_oB = ctx.enter_context(tc.tile_pool(name="ps_oB", bufs=1, space="PSUM"))
    dram = ctx.enter_context(tc.tile_pool(name="dram", bufs=1, space="DRAM"))

    def load_const(name, shape, dt=F32):
        t = const.tile(list(shape), dt, tag=name, name=name)
        nc.sync.dma_start(t[:], io[name][:])
        return t

    W0h = load_const("W0h", (2, H), F16)
    b0n = load_const("b0n", (H, 1))
    W1 = load_const("W1", (H, H), F16)
    W1x = load_const("W1x", (H, H), F16)
    W1xx = load_const("W1xx", (H, H), F16)
    W1xxx = load_const("W1xxx", (H, H), F16)
    W1t = load_const("W1t", (H, H), F16)
    b1 = load_const("b1", (H, 1))
    W2 = load_const("W2", (H, H), F16)
    b2 = load_const("b2", (H, 1))
    W3s = load_const("W3s", (H, 10), F16)
    b3u = load_const("b3u", (128, 1))
    b3c = load_const("b3c", (128, 1))
    bqh = load_const("bqh", (128, 1))
    XiA_sb = load_const("XiA", (2, NI), F16)
    XiB_sb = load_const("XiB", (2, NI), F16)
    XlA_sb = load_const("XlA", (2, NB), F16)
    XlB_sb = load_const("XlB", (2, NB), F16)
    XrA_sb = load_const("XrA", (2, NB), F16)
    XrB_sb = load_const("XrB", (2, NB), F16)
    X0A_sb = load_const("X0A", (2, NB), F16)
    X0B_sb = load_const("X0B", (2, NB), F16)
    x_pm = load_const("x_pm", (128, PI))
    xl_pm = load_const("xl_pm", (128, PB))
    xr_pm = load_const("xr_pm", (128, PB))
    Wi0 = load_const("Wi0", (128, 2, PB))
    Yi0 = load_const("Yi0", (128, 2, PB))

    KDBG = int(os.environ.get("KDBG", "0"))

    def ctile(tag):
        return chan.tile([128, B], F16, tag=tag, name=tag)

    def ttile(tag):
        return tmp.tile([128, B], F16, tag=tag, name=tag)

    def layer1(XA, XB, k, nch):
        """First layer from split fp16 coords.  Returns rhs tiles for the
        diag-folded layer-2 matmuls: [y, p, yp, pq, p] (x/t channels both
        use p; xx uses yp; xxx uses pq)."""
        z0 = ps_z.tile([128, B], F32, tag="zv", name="z0")
        nc.tensor.matmul(z0[:], lhsT=W0h[:], rhs=XA[:, k * B : (k + 1) * B],
                         start=True, stop=False)
        nc.tensor.matmul(z0[:], lhsT=W0h[:], rhs=XB[:, k * B : (k + 1) * B],
                         start=False, stop=True)
        y = ctile("h_v")
        nc.scalar.activation(y[:], z0[:], AF.Tanh, bias=b0n[:], scale=1.0)
        if nch == 1:
            return [y[:]]
        ysq = ttile("ysq")
        nc.scalar.activation(ysq[:], y[:], AF.Square)
        p = ctile("h_p")
        nc.vector.tensor_scalar(p[:], ysq[:], -1.0, 1.0, OP.mult, OP.add)
        yp = ctile("h_yp")
        nc.vector.tensor_mul(yp[:], y[:], p[:])
        if nch == 3:
            return [y[:], p[:], yp[:]]
        qh = ttile("qh")
        nc.vector.tensor_scalar(qh[:], ysq[:], 0.5, -1.0 / 6.0, OP.mult, OP.add)
        pq = ctile("h_pq")
        nc.vector.tensor_mul(pq[:], p[:], qh[:])
        return [y[:], p[:], yp[:], pq[:], p[:]]

    def hidden(chs, Ws, b, nch):
        """One hidden tanh layer, rescaled derivative propagation (16-bit).

        Channel scalings: x: s1=1, xx: s2=-1/2, xxx: s3=1/12, t: 1.  With
        qh = ysq/2 - 1/6 the chain rule is constant-free:
          y1 = p*z1;  y2 = p*(z2 + y*z1^2);  y3 = p*(z3 + z1*(y*z2 + qh*z1^2))
        """
        Wv, Wx, Wxx, Wxxx, Wt = Ws
        zv = ps_z.tile([128, B], F32, tag="zv", name="zv")
        nc.tensor.matmul(zv[:], lhsT=Wv[:], rhs=chs[0])
        y = ctile("h_v")
        nc.scalar.activation(y[:], zv[:], AF.Tanh, bias=b[:], scale=1.0)
        if nch == 1:
            return [y[:]]
        zxt = ps_z.tile([128, 2 * B], F32, tag="zxt", name="zxt")
        nc.tensor.matmul(zxt[:, 0:B], lhsT=Wx[:], rhs=chs[1])
        zxx = ps_z.tile([128, B], F32, tag="zxx", name="zxx")
        nc.tensor.matmul(zxx[:], lhsT=Wxx[:], rhs=chs[2])
        if nch == 5:
            nc.tensor.matmul(zxt[:, B : 2 * B], lhsT=Wt[:], rhs=chs[4])
            zxxx = ps_z.tile([128, B], F32, tag="zxxx", name="zxxx")
            nc.tensor.matmul(zxxx[:], lhsT=Wxxx[:], rhs=chs[3])
        z1c = ttile("z1c")
        nc.scalar.copy(z1c[:], zxt[:, 0:B])
        ysq = ttile("ysq")
        nc.scalar.activation(ysq[:], y[:], AF.Square)
        p = ttile("p")
        nc.scalar.activation(p[:], ysq[:], AF.Identity, bias=1.0, scale=-1.0)
        z1sq = ttile("z1sq")
        nc.vector.tensor_mul(z1sq[:], z1c[:], z1c[:])
        y1 = ctile("h_x")
        nc.vector.tensor_mul(y1[:], p[:], z1c[:])
        g = ttile("g")
        nc.vector.tensor_mul(g[:], y[:], z1sq[:])
        if nch == 3:
            m2 = ttile("m2")
            nc.vector.tensor_add(m2[:], g[:], zxx[:])
            y2 = ctile("h_xx")
            nc.vector.tensor_mul(y2[:], p[:], m2[:])
            return [y[:], y1[:], y2[:]]
        z2c = ttile("z2c")
        nc.scalar.copy(z2c[:], zxx[:])
        ztc = ttile("ztc")
        nc.scalar.copy(ztc[:], zxt[:, B : 2 * B])
        yt = ctile("h_t")
        nc.vector.tensor_mul(yt[:], p[:], ztc[:])
        m2 = ttile("m2")
        nc.vector.tensor_add(m2[:], g[:], z2c[:])
        y2 = ctile("h_xx")
        nc.vector.tensor_mul(y2[:], p[:], m2[:])
        qh = ttile("qh")
        nc.scalar.activation(qh[:], ysq[:], AF.Identity, bias=bqh[:],
                             scale=0.5)
        w1 = ttile("w1")
        nc.vector.tensor_mul(w1[:], y[:], z2c[:])
        u1 = ttile("u1")
        nc.vector.tensor_mul(u1[:], qh[:], z1sq[:])
        n3 = ttile("n3")
        nc.vector.tensor_add(n3[:], w1[:], u1[:])
        n4 = ttile("n4")
        nc.vector.tensor_mul(n4[:], n3[:], z1c[:])
        n5 = ttile("n5")
        nc.vector.tensor_add(n5[:], n4[:], zxxx[:])
        y3 = ctile("h_xxx")
        nc.vector.tensor_mul(y3[:], p[:], n5[:])
        return [y[:], y1[:], y2[:], y3[:], yt[:]]

    def emit_block(XA, XB, k, nch, Ud):
        """One 512-point block of a point set, through the whole net."""
        chs = layer1(XA, XB, k, nch)
        W1s = (W1, W1x, W1xx, W1xxx, W1t)
        W2s = (W2, W2, W2, W2, W2)
        chs = hidden(chs, W1s, b1, nch)
        chs = hidden(chs, W2s, b2, nch)
        # out projections at partition bases 0/32/64 of a shared bank; one
        # cast per bank then strided DMAs pick the populated rows.
        nA = min(nch, 3)
        ocA = ps_oA.tile([128, B], F32, tag="ocA", name="ocA")
        for c in range(nA):
            nc.tensor.matmul(ocA[32 * c : 32 * c + 2, :],
                             lhsT=W3s[:, 2 * c : 2 * c + 2], rhs=chs[c])
        if nch == 5:
            ocB = ps_oB.tile([128, B], F32, tag="ocB", name="ocB")
            for j, c in enumerate((3, 4)):
                nc.tensor.matmul(ocB[32 * j : 32 * j + 2, :],
                                 lhsT=W3s[:, 2 * c : 2 * c + 2], rhs=chs[c])
        col = slice(k * B, (k + 1) * B)
        if nch == 1:
            ob = tmp.tile([2, B], F16, tag="ob1", name="ob")
            nc.vector.tensor_copy(ob[:], ocA[0:2, :])
            nc.sync.dma_start(Ud[0:2, col], ob[0:2, :])
            return
        obA = tmp.tile([66, B], F16, tag="obA", name="obA")
        if k % 2 == 0:
            nc.vector.tensor_copy(obA[:], ocA[0:66, :])
        else:
            nc.scalar.copy(obA[:], ocA[0:66, :])
        # u-parts: rows u,ux,uxx; c-parts: c,cx(,cxx)
        nc.sync.dma_start(Ud[0:3, col], obA[0:65:32, :])
        if nch == 3:
            nc.sync.dma_start(Ud[3:5, col], obA[1:34:32, :])
            return
        nc.sync.dma_start(Ud[4:7, col], obA[1:66:32, :])
        obB = tmp.tile([34, B], F16, tag="obB", name="obB")
        nc.vector.tensor_copy(obB[:], ocB[0:34, :])
        nc.sync.dma_start(Ud[3:4, col], obB[0:1, :])
        nc.sync.dma_start(Ud[7:8, col], obB[33:34, :])

    # ---------------- point sets (interleaved schedule) ----------------
    Ud_i = dram.tile([8, NI], F16, tag="Ud_i")
    Ud_l = dram.tile([5, NB], F16, tag="Ud_l")
    Ud_r = dram.tile([5, NB], F16, tag="Ud_r")
    Ud_0 = dram.tile([2, NB], F16, tag="Ud_0")
    bnd_stage = {"l": Ud_l, "r": Ud_r}
    jobs = []
    small = [("l", 0), ("l", 1), ("r", 0), ("r", 1), ("0", 0), ("0", 1)]
    for k in range(NBI):
        jobs.append(("i", k))
        if k % 3 == 2 and small:
            jobs.append(small.pop(0))
    jobs += small
    if KDBG in (2, 3):
        jobs = []
    for set_, k in jobs:
        if set_ == "i":
            emit_block(XiA_sb, XiB_sb, k, 5, Ud_i)
        elif set_ == "l":
            emit_block(XlA_sb, XlB_sb, k, 3, Ud_l)
        elif set_ == "r":
            emit_block(XrA_sb, XrB_sb, k, 3, Ud_r)
        else:
            emit_block(X0A_sb, X0B_sb, k, 1, Ud_0)

    # ---------------- output accumulator ----------------
    out_sb = res.tile([128, OUT_COLS], F32, tag="out_sb")
    if KDBG in (1, 3):
        nc.vector.memset(out_sb[:], 0.0)
        nc.sync.dma_start(io["out"][:], out_sb[:])
        return

    def rt(tag, w=PI):
        return res.tile([128, w], F32, tag=tag, name=tag)

    def tt(out, a, b_, op=OP.mult):
        nc.vector.tensor_tensor(as_ap(out), as_ap(a), as_ap(b_), op)
        return out

    def stt(out, in0, scal, in1, op0=OP.mult, op1=OP.add):
        nc.vector.scalar_tensor_tensor(as_ap(out), as_ap(in0), scal,
                                       as_ap(in1), op0, op1)
        return out

    def sumsq(src, colidx, scale=1.0, w=PI):
        dump = res.tile([128, w], F32, tag="dump", name="dump")
        nc.vector.tensor_tensor(dump[:], as_ap(src), as_ap(src), OP.mult)
        nc.vector.tensor_scalar(
            dump[:], dump[:], float(scale), 0.0, OP.mult, OP.add,
            accum_out=out_sb[:, colidx : colidx + 1])

    # ---------------- boundary residuals ----------------
    for si, (setname, xp, rhs) in enumerate((("l", xl_pm, 0.0), ("r", xr_pm, 1.0))):
        Ub = res.tile([128, 5, PB], F16, tag="Ub", name="Ub")
        nc.sync.dma_start(Ub[:], bnd_stage[setname][:].rearrange(
            "c (p i) -> p c i", p=128))
        ub_ = res.tile([128, PB], F32, tag="bu", name="bu")
        nc.vector.tensor_scalar(ub_[:], Ub[:, BR_U, :], b3u[:], None, OP.add)
        cb = res.tile([128, PB], F32, tag="bc", name="bc")
        nc.vector.tensor_scalar(cb[:], Ub[:, BR_C, :], b3c[:], None, OP.add)
        uxb, cxb, uxxb = Ub[:, BR_UX, :], Ub[:, BR_CX, :], Ub[:, BR_UXX, :]
        bx2 = res.tile([128, PB], F32, tag="bx2", name="bx2")
        nc.scalar.activation(bx2[:], xp[:], AF.Square)
        rx = res.tile([128, PB], F32, tag="rx", name="rx")
        nc.vector.reciprocal(rx[:], xp[:])
        t1 = res.tile([128, PB], F32, tag="t1", name="t1")
        tt(t1, ub_, rx)
        t2 = res.tile([128, PB], F32, tag="t2", name="t2")
        nc.vector.tensor_scalar(t2[:], t1[:], LAM3, None, OP.mult)
        t3 = res.tile([128, PB], F32, tag="t3", name="t3")
        nc.vector.scalar_tensor_tensor(t3[:], uxb, LAM2, t2[:], OP.mult, OP.add)
        t4 = res.tile([128, PB], F32, tag="t4", name="t4")
        nc.vector.scalar_tensor_tensor(t4[:], cb[:], -ALFA / 3.0, t3[:],
                                       OP.mult, OP.add)
        sumsq(t4, 2 + 2 * si, scale=LAM1 * LAM1, w=PB)
        m1b = res.tile([128, PB], F32, tag="m1b", name="m1b")
        nc.vector.tensor_tensor(m1b[:], bx2[:], uxxb, OP.mult)
        m2b = res.tile([128, PB], F32, tag="m2b", name="m2b")
        nc.vector.tensor_tensor(m2b[:], xp[:], uxb, OP.mult)
        m3b = res.tile([128, PB], F32, tag="m3b", name="m3b")
        tt(m3b, m1b, m2b, OP.add)
        m4b = res.tile([128, PB], F32, tag="m4b", name="m4b")
        tt(m4b, m3b, ub_, OP.subtract)
        Cb = res.tile([128, PB], F32, tag="Cb", name="Cb")
        nc.vector.tensor_tensor(Cb[:], bx2[:], cxb, OP.mult)
        m5b = res.tile([128, PB], F32, tag="m5b", name="m5b")
        stt(m5b, Cb, -ALFA, m4b)
        m6b = res.tile([128, PB], F32, tag="m6b", name="m6b")
        tt(m6b, cb, m5b)
        fd = res.tile([128, PB], F32, tag="fd", name="fd")
        stt(fd, m6b, -THETA, Cb)
        if rhs != 0.0:
            fd2 = res.tile([128, PB], F32, tag="fd2", name="fd2")
            tt(fd2, fd, bx2, OP.subtract)
            fd = fd2
        sumsq(fd, 3 + 2 * si, w=PB)

    # ---------------- init residuals ----------------
    U0 = res.tile([128, 2, PB], F16, tag="U0")
    nc.sync.dma_start(U0[:], Ud_0[:].rearrange("c (p i) -> p c i", p=128))
    iu = res.tile([128, PB], F32, tag="iu")
    nc.vector.tensor_scalar(iu[:], U0[:, 0, :], b3u[:], None, OP.add)
    ic = res.tile([128, PB], F32, tag="ic")
    nc.vector.tensor_scalar(ic[:], U0[:, 1, :], b3c[:], None, OP.add)
    for oi, val in enumerate((iu, ic)):
        for j in range(2):
            d = res.tile([128, PB], F32, tag="d0", name="d0")
            nc.vector.tensor_tensor(d[:], val[:], Yi0[:, j, :], OP.subtract)
            dw = res.tile([128, PB], F32, tag="dw", name="dw")
            nc.vector.tensor_tensor(dw[:], d[:], Wi0[:, j, :], OP.mult)
            dump = res.tile([128, PB], F32, tag="dump0", name="dump0")
            nc.vector.tensor_tensor(dump[:], d[:], dw[:], OP.mult)
            nc.vector.tensor_scalar(
                dump[:], dump[:], 1.0, 0.0, OP.mult, OP.add,
                accum_out=out_sb[:, 6 + 2 * oi + j : 7 + 2 * oi + j])

    # ---------------- interior residuals ----------------
    Ui = res.tile([128, 8, PI], F16, tag="Ui")
    nc.sync.dma_start(Ui[:], Ud_i[:].rearrange("c (p i) -> p c i", p=128))
    if KDBG == 5:
        nc.vector.memset(out_sb[:], 0.0)
        nc.vector.tensor_copy(out_sb[:, 0:8], Ui[:, :, 0])
        nc.sync.dma_start(io["out"][:], out_sb[:])
        return
    ub_t = rt("ub_t")
    nc.vector.tensor_scalar(ub_t[:], Ui[:, IR_U, :], b3u[:], None, OP.add)
    cb_t = rt("cb_t")
    nc.vector.tensor_scalar(cb_t[:], Ui[:, IR_C, :], b3c[:], None, OP.add)
    x = x_pm
    x2 = rt("x2")
    nc.scalar.activation(x2[:], x[:], AF.Square)
    x3 = tt(rt("x3"), x2, x)
    A = tt(rt("A"), x2[:], Ui[:, IR_UXX, :])          # x2*uxx
    Bt = tt(rt("Bt"), x[:], Ui[:, IR_UX, :])          # x*ux
    C = tt(rt("C"), x2[:], Ui[:, IR_CX, :])           # x2*cx
    D = tt(rt("D"), x3[:], Ui[:, IR_CXX, :])          # x3*cxx
    E = tt(rt("E"), x3[:], Ui[:, IR_UXXX, :])         # x3*uxxx
    F = tt(rt("F"), x3[:], Ui[:, IR_CT, :])           # x3*ct
    j1 = tt(rt("j1"), A, Bt, OP.add)
    j2 = tt(rt("j2"), j1, ub_t, OP.subtract)     # A+B-u
    fs = stt(rt("fs"), C, -K1, j2)               # -K1*C + j2
    sumsq(fs, 0)
    in2 = stt(rt("in2"), C, -ALFA, j2)           # inner2
    i1 = stt(rt("i1"), A, 2.0, Bt, OP.mult, OP.subtract)   # 2A - B
    i2 = tt(rt("i2"), i1, ub_t, OP.add)
    i3 = stt(rt("i3"), C, -ALFA, E)
    i4 = tt(rt("i4"), i2, i3, OP.add)
    i5 = stt(rt("i5"), D, -ALFA, i4)             # inner1
    k1 = tt(rt("k1"), cb_t, i5)
    k2 = tt(rt("k2"), x, Ui[:, IR_CX, :])        # x*cx
    k3 = tt(rt("k3"), k2, in2)
    k4 = stt(rt("k4"), k1, THETA, F)
    k5 = stt(rt("k5"), k3, THETA, k4)
    k7 = tt(rt("k7"), D, C, OP.add)
    r_ = tt(rt("r_"), k5, k7, OP.subtract)
    sumsq(r_, 1)

    nc.sync.dma_start(io["out"][:], out_sb[:])


def build_nc():
    nc = bacc.Bacc("TRN2", target_bir_lowering=False, debug=False, num_devices=M)
    io = {}

    def dp(name, shape, is_out=False, dt=F32):
        h = nc.declare_dram_parameter(name, list(shape), dt, isOutput=is_out)
        io[name] = h.ap()

    dp("XiA", (2, NI), dt=F16); dp("XiB", (2, NI), dt=F16)
    dp("XlA", (2, NB), dt=F16); dp("XlB", (2, NB), dt=F16)
    dp("XrA", (2, NB), dt=F16); dp("XrB", (2, NB), dt=F16)
    dp("X0A", (2, NB), dt=F16); dp("X0B", (2, NB), dt=F16)
    dp("x_pm", (128, PI)); dp("xl_pm", (128, PB)); dp("xr_pm", (128, PB))
    dp("W0h", (2, H), dt=F16); dp("b0n", (H, 1))
    dp("W1", (H, H), dt=F16); dp("b1", (H, 1))
    dp("W1x", (H, H), dt=F16); dp("W1xx", (H, H), dt=F16)
    dp("W1xxx", (H, H), dt=F16); dp("W1t", (H, H), dt=F16)
    dp("W2", (H, H), dt=F16); dp("b2", (H, 1))
    dp("W3s", (H, 10), dt=F16); dp("b3u", (128, 1)); dp("b3c", (128, 1))
    dp("bqh", (128, 1))
    dp("Wi0", (128, 2, PB)); dp("Yi0", (128, 2, PB))
    dp("out", (128, OUT_COLS), is_out=True)

    with tile.TileContext(nc) as tc:
        with ExitStack() as ctx:
            emit(nc, tc, ctx, io)
    nc.compile()
    return nc


def host_prep(inputs):
    """Fold normalization into layer-0 weights and build the 8 per-core maps."""
    import ml_dtypes
    f4, f2 = np.float32, ml_dtypes.bfloat16
    g = {k: np.asarray(v) for k, v in inputs.items()}
    Xint = g["Xint"].astype(f4)
    lb = Xint.min(axis=0).astype(np.float64)
    ub = Xint.max(axis=0).astype(np.float64)
    a = 2.0 / (ub - lb)
    W0 = g["W0"].astype(np.float64)
    W0n = a[:, None] * W0
    beta = -2.0 * lb / (ub - lb) - 1.0
    b0n = beta @ W0 + g["b0"].astype(np.float64)
    W0h = W0n.astype(f2)
    w0x = W0h[0].astype(np.float64)   # match device: fp16 weight actually used

    # diag-folded layer-2 weights (channel scalings s1=1, s2=-1/2, s3=1/12
    # are folded here and un-done per-channel in W3s)
    w0t = W0h[1].astype(np.float64)
    W1f = g["W1"].astype(np.float64)
    W3f = g["W3"].astype(np.float64)
    W3sc = np.concatenate([W3f, W3f, -2.0 * W3f, 12.0 * W3f, W3f], axis=1)
    com = {
        "W0h": W0h,
        "b0n": b0n.astype(f4).reshape(H, 1),
        "W1": g["W1"].astype(f2), "b1": g["b1"].astype(f4).reshape(H, 1),
        "W1x": (w0x[:, None] * W1f).astype(f2),
        "W1xx": (w0x[:, None] ** 2 * W1f).astype(f2),
        "W1xxx": (w0x[:, None] ** 3 * W1f).astype(f2),
        "W1t": (w0t[:, None] * W1f).astype(f2),
        "W2": g["W2"].astype(f2), "b2": g["b2"].astype(f4).reshape(H, 1),
        "W3s": W3sc.astype(f2),
        "b3u": np.full((128, 1), g["b3"][0], f4),
        "b3c": np.full((128, 1), g["b3"][1], f4),
        "bqh": np.full((128, 1), -1.0 / 6.0, f4),
    }

    def split16(X):  # [n,2] fp32 -> (hi, lo) fp16 transposed [2,n]
        XT = np.ascontiguousarray(X.T.astype(f4))
        A = XT.astype(f2)
        Bx = (XT - A.astype(f4)).astype(f2)
        return A, Bx

    in_maps = []
    for c in range(M):
        Xi = Xint[c * NI : (c + 1) * NI]
        Xl = g["Xbnd_l"][c * NB : (c + 1) * NB].astype(f4)
        Xr = g["Xbnd_r"][c * NB : (c + 1) * NB].astype(f4)
        X0 = g["Xinit"][c * NB : (c + 1) * NB].astype(f4)
        Wi = g["Winit"][c * NB : (c + 1) * NB].astype(f4)
        Yi = g["Yinit"][c * NB : (c + 1) * NB].astype(f4)
        m = dict(com)
        m["XiA"], m["XiB"] = split16(Xi)
        m["XlA"], m["XlB"] = split16(Xl)
        m["XrA"], m["XrB"] = split16(Xr)
        m["X0A"], m["X0B"] = split16(X0)
        m["x_pm"] = np.ascontiguousarray(Xi[:, 0].reshape(128, PI))
        m["xl_pm"] = np.ascontiguousarray(Xl[:, 0].reshape(128, PB))
        m["xr_pm"] = np.ascontiguousarray(Xr[:, 0].reshape(128, PB))
        m["Wi0"] = np.ascontiguousarray(Wi.reshape(128, PB, 2).transpose(0, 2, 1))
        m["Yi0"] = np.ascontiguousarray(Yi.reshape(128, PB, 2).transpose(0, 2, 1))
        in_maps.append(m)
    return in_maps


def combine(results):
    s = np.zeros(OUT_COLS, np.float64)
    for r in results:
        s += r["out"].astype(np.float64).sum(axis=0)
    int_loss = (s[0] + s[1]) / N_INT
    bnd_loss = (s[2] + s[3]) / N_BND + (s[4] + s[5]) / N_BND
    init_loss = (s[6] + s[7] + s[8] + s[9]) / (2 * N_BND)
    return np.float32(int_loss + bnd_loss + init_loss)


_CACHE = {}


def _get_nc():
    if "nc" not in _CACHE:
        _CACHE["nc"] = build_nc()
    return _CACHE["nc"]


def kernel(**inputs):
    in_maps = host_prep(inputs)
    nc = _get_nc()
    res = run_bass_kernel_spmd(nc, in_maps, core_ids=list(range(M)))
    return combine(res.results)


# revision 28
# speedup vs baseline: 1.2092x; 1.0055x over previous
"""Trainium2 Bass kernel: Diffusion-DEM PINN total loss (data-parallel, 8 cores).

Strategy
--------
Pure data parallel: every collocation-point set is sharded along axis 0 over
the 8 NeuronCores; the small MLP (2->128->128->128->2, tanh) is replicated.
Derivatives (u,c, ux,cx, ct, uxx,cxx, uxxx) are propagated analytically in
forward mode: each tanh layer carries 5 channels (value, d/dx, d2/dx2,
d3/dx3, d/dt).  The whole channel pipeline runs in fp16 (weights, channels,
chain-rule algebra) -> tensor engine at full rate and vector engine in
2x/4x perf modes; matmul accumulation and the final residual reduction stay
fp32.  Layer-0 input is fed as an exact hi+lo fp16 pair (two accumulating
matmuls) so collocation coordinates keep fp32 accuracy.  Per-point outputs
are staged feature-major to DRAM in fp16 (all out-projections share one
PSUM bank at different partition offsets -> one copy + one DMA per block),
gathered back point-major, and the PDE residuals + squared sums are
computed on-chip in fp32.  Each core returns [128, 10] partial sums; the
host combines them into the scalar loss.

Channel order everywhere: v=0, d/dx=1, d2/dx2=2, d3/dx3=3, d/dt=4.
"""

import os
import numpy as np
from contextlib import ExitStack

import concourse.bass as bass
import concourse.tile as tile
import concourse.mybir as mybir
from concourse import bacc
from concourse.bass_utils import run_bass_kernel_spmd

F32 = mybir.dt.float32
F16 = mybir.dt.float16
AF = mybir.ActivationFunctionType
OP = mybir.AluOpType

M = 8                      # cores
N_INT, N_BND, H = 65536, 8192, 128
NI = N_INT // M            # 8192 interior pts / core
NB = N_BND // M            # 1024 bnd / init pts / core
B = 512                    # point block (PSUM bank width, fp32)
NBI, NBB = NI // B, NB // B
PI, PB = NI // 128, NB // 128   # 64, 8 point-major free dims

# physical constants (from reference)
NU = 0.3
ALFA = 0.001 * 8e-07 * 3.497e-06 / 7.08e-15
THETA = 3.497e-06 * 1e10 / (8.3145 * 300.0 * 3.0 * 0.4)
LAM1 = 1.0 / (1.0 - 2.0 * NU)
LAM2 = (1.0 - NU) / (1.0 + NU)
LAM3 = NU / (1.0 + NU)
K1 = ALFA * 1.3 / (0.7 * 3.0)

OUT_COLS = 10  # fs2, resid2, stress_l2, fd_l2, stress_r2, fd_r2, iu0, iu1, ic0, ic1

# interior staging rows (fp16, [8, NI]): u,ux,uxx,uxxx then c,cx,cxx,ct
IR_U, IR_UX, IR_UXX, IR_UXXX, IR_C, IR_CX, IR_CXX, IR_CT = range(8)
# boundary staging rows (fp16, [5, NB]): u,ux,uxx then c,cx
BR_U, BR_UX, BR_UXX, BR_C, BR_CX = range(5)


def as_ap(v):
    return v if isinstance(v, bass.AP) else v[:]


def emit(nc, tc, ctx, io):
    """Emit the whole per-core program. io maps name -> bass AP (dram)."""
    const = ctx.enter_context(tc.tile_pool(name="const", bufs=1))
    chan = ctx.enter_context(tc.tile_pool(name="chan", bufs=5))
    tmp = ctx.enter_context(tc.tile_pool(name="tmp", bufs=5))
    res = ctx.enter_context(tc.tile_pool(name="res", bufs=1))
    ps_z = ctx.enter_context(tc.tile_pool(name="ps_z", bufs=1, space="PSUM"))
    ps_oA = ctx.enter_context(tc.tile_pool(name="ps_oA", bufs=2, space="PSUM"))
    ps_oB = ctx.enter_context(tc.tile_pool(name="ps_oB", bufs=1, space="PSUM"))
    dram = ctx.enter_context(tc.tile_pool(name="dram", bufs=1, space="DRAM"))

    def load_const(name, shape, dt=F32):
        t = const.tile(list(shape), dt, tag=name, name=name)
        nc.sync.dma_start(t[:], io[name][:])
        return t

    W0h = load_const("W0h", (2, H), F16)
    b0n = load_const("b0n", (H, 1))
    W1 = load_const("W1", (H, H), F16)
    W1x = load_const("W1x", (H, H), F16)
    W1xx = load_const("W1xx", (H, H), F16)
    W1xxx = load_const("W1xxx", (H, H), F16)
    W1t = load_const("W1t", (H, H), F16)
    b1 = load_const("b1", (H, 1))
    W2 = load_const("W2", (H, H), F16)
    b2 = load_const("b2", (H, 1))
    W3s = load_const("W3s", (H, 10), F16)
    b3u = load_const("b3u", (128, 1))
    b3c = load_const("b3c", (128, 1))
    bqh = load_const("bqh", (128, 1))
    XiA_sb = load_const("XiA", (2, NI), F16)
    XiB_sb = load_const("XiB", (2, NI), F16)
    XlA_sb = load_const("XlA", (2, NB), F16)
    XlB_sb = load_const("XlB", (2, NB), F16)
    XrA_sb = load_const("XrA", (2, NB), F16)
    XrB_sb = load_const("XrB", (2, NB), F16)
    X0A_sb = load_const("X0A", (2, NB), F16)
    X0B_sb = load_const("X0B", (2, NB), F16)
    x_pm = load_const("x_pm", (128, PI))
    xl_pm = load_const("xl_pm", (128, PB))
    xr_pm = load_const("xr_pm", (128, PB))
    Wi0 = load_const("Wi0", (128, 2, PB))
    Yi0 = load_const("Yi0", (128, 2, PB))

    KDBG = int(os.environ.get("KDBG", "0"))

    def ctile(tag):
        return chan.tile([128, B], F16, tag=tag, name=tag)

    def ttile(tag):
        return tmp.tile([128, B], F16, tag=tag, name=tag)

    def layer1(XA, XB, k, nch):
        """First layer from split fp16 coords.  Returns rhs tiles for the
        diag-folded layer-2 matmuls: [y, p, yp, pq, p] (x/t channels both
        use p; xx uses yp; xxx uses pq)."""
        z0 = ps_z.tile([128, B], F32, tag="zv", name="z0")
        nc.tensor.matmul(z0[:], lhsT=W0h[:], rhs=XA[:, k * B : (k + 1) * B],
                         start=True, stop=False)
        nc.tensor.matmul(z0[:], lhsT=W0h[:], rhs=XB[:, k * B : (k + 1) * B],
                         start=False, stop=True)
        y = ctile("h_v")
        nc.scalar.activation(y[:], z0[:], AF.Tanh, bias=b0n[:], scale=1.0)
        if nch == 1:
            return [y[:]]
        ysq = ttile("ysq")
        nc.scalar.activation(ysq[:], y[:], AF.Square)
        p = ctile("h_p")
        nc.vector.tensor_scalar(p[:], ysq[:], -1.0, 1.0, OP.mult, OP.add)
        yp = ctile("h_yp")
        nc.vector.tensor_mul(yp[:], y[:], p[:])
        if nch == 3:
            return [y[:], p[:], yp[:]]
        qh = ttile("qh")
        nc.vector.tensor_scalar(qh[:], ysq[:], 0.5, -1.0 / 6.0, OP.mult, OP.add)
        pq = ctile("h_pq")
        nc.vector.tensor_mul(pq[:], p[:], qh[:])
        return [y[:], p[:], yp[:], pq[:], p[:]]

    def hidden(chs, Ws, b, nch):
        """One hidden tanh layer, rescaled derivative propagation (16-bit).

        Channel scalings: x: s1=1, xx: s2=-1/2, xxx: s3=1/12, t: 1.  With
        qh = ysq/2 - 1/6 the chain rule is constant-free:
          y1 = p*z1;  y2 = p*(z2 + y*z1^2);  y3 = p*(z3 + z1*(y*z2 + qh*z1^2))
        """
        Wv, Wx, Wxx, Wxxx, Wt = Ws
        zv = ps_z.tile([128, B], F32, tag="zv", name="zv")
        nc.tensor.matmul(zv[:], lhsT=Wv[:], rhs=chs[0])
        y = ctile("h_v")
        nc.scalar.activation(y[:], zv[:], AF.Tanh, bias=b[:], scale=1.0)
        if nch == 1:
            return [y[:]]
        zxt = ps_z.tile([128, 2 * B], F32, tag="zxt", name="zxt")
        nc.tensor.matmul(zxt[:, 0:B], lhsT=Wx[:], rhs=chs[1])
        zxx = ps_z.tile([128, B], F32, tag="zxx", name="zxx")
        nc.tensor.matmul(zxx[:], lhsT=Wxx[:], rhs=chs[2])
        if nch == 5:
            nc.tensor.matmul(zxt[:, B : 2 * B], lhsT=Wt[:], rhs=chs[4])
            zxxx = ps_z.tile([128, B], F32, tag="zxxx", name="zxxx")
            nc.tensor.matmul(zxxx[:], lhsT=Wxxx[:], rhs=chs[3])
        z1c = ttile("z1c")
        nc.scalar.copy(z1c[:], zxt[:, 0:B])
        ysq = ttile("ysq")
        nc.scalar.activation(ysq[:], y[:], AF.Square)
        p = ttile("p")
        nc.scalar.activation(p[:], ysq[:], AF.Identity, bias=1.0, scale=-1.0)
        z1sq = ttile("z1sq")
        nc.vector.tensor_mul(z1sq[:], z1c[:], z1c[:])
        y1 = ctile("h_x")
        nc.vector.tensor_mul(y1[:], p[:], z1c[:])
        g = ttile("g")
        nc.vector.tensor_mul(g[:], y[:], z1sq[:])
        if nch == 3:
            m2 = ttile("m2")
            nc.vector.tensor_add(m2[:], g[:], zxx[:])
            y2 = ctile("h_xx")
            nc.vector.tensor_mul(y2[:], p[:], m2[:])
            return [y[:], y1[:], y2[:]]
        z2c = ttile("z2c")
        nc.scalar.copy(z2c[:], zxx[:])
        ztc = ttile("ztc")
        nc.scalar.copy(ztc[:], zxt[:, B : 2 * B])
        yt = ctile("h_t")
        nc.vector.tensor_mul(yt[:], p[:], ztc[:])
        m2 = ttile("m2")
        nc.vector.tensor_add(m2[:], g[:], z2c[:])
        y2 = ctile("h_xx")
        nc.vector.tensor_mul(y2[:], p[:], m2[:])
        qh = ttile("qh")
        nc.scalar.activation(qh[:], ysq[:], AF.Identity, bias=bqh[:],
                             scale=0.5)
        w1 = ttile("w1")
        nc.vector.tensor_mul(w1[:], y[:], z2c[:])
        u1 = ttile("u1")
        nc.vector.tensor_mul(u1[:], qh[:], z1sq[:])
        n3 = ttile("n3")
        nc.vector.tensor_add(n3[:], w1[:], u1[:])
        n4 = ttile("n4")
        nc.vector.tensor_mul(n4[:], n3[:], z1c[:])
        n5 = ttile("n5")
        nc.vector.tensor_add(n5[:], n4[:], zxxx[:])
        y3 = ctile("h_xxx")
        nc.vector.tensor_mul(y3[:], p[:], n5[:])
        return [y[:], y1[:], y2[:], y3[:], yt[:]]

    def emit_block(XA, XB, k, nch, Ud):
        """One 512-point block of a point set, through the whole net."""
        chs = layer1(XA, XB, k, nch)
        W1s = (W1, W1x, W1xx, W1xxx, W1t)
        W2s = (W2, W2, W2, W2, W2)
        chs = hidden(chs, W1s, b1, nch)
        chs = hidden(chs, W2s, b2, nch)
        # out projections at partition bases 0/32/64 of a shared bank; one
        # cast per bank then strided DMAs pick the populated rows.
        nA = min(nch, 3)
        ocA = ps_oA.tile([128, B], F32, tag="ocA", name="ocA")
        for c in range(nA):
            nc.tensor.matmul(ocA[32 * c : 32 * c + 2, :],
                             lhsT=W3s[:, 2 * c : 2 * c + 2], rhs=chs[c])
        if nch == 5:
            ocB = ps_oB.tile([128, B], F32, tag="ocB", name="ocB")
            for j, c in enumerate((3, 4)):
                nc.tensor.matmul(ocB[32 * j : 32 * j + 2, :],
                                 lhsT=W3s[:, 2 * c : 2 * c + 2], rhs=chs[c])
        col = slice(k * B, (k + 1) * B)
        if nch == 1:
            ob = tmp.tile([2, B], F16, tag="ob1", name="ob")
            nc.vector.tensor_copy(ob[:], ocA[0:2, :])
            nc.sync.dma_start(Ud[0:2, col], ob[0:2, :])
            return
        obA = tmp.tile([66, B], F16, tag="obA", name="obA")
        if k % 2 == 0:
            nc.vector.tensor_copy(obA[:], ocA[0:66, :])
        else:
            nc.scalar.copy(obA[:], ocA[0:66, :])
        # u-parts: rows u,ux,uxx; c-parts: c,cx(,cxx)
        nc.sync.dma_start(Ud[0:3, col], obA[0:65:32, :])
        if nch == 3:
            nc.sync.dma_start(Ud[3:5, col], obA[1:34:32, :])
            return
        nc.sync.dma_start(Ud[4:7, col], obA[1:66:32, :])
        obB = tmp.tile([34, B], F16, tag="obB", name="obB")
        nc.vector.tensor_copy(obB[:], ocB[0:34, :])
        nc.sync.dma_start(Ud[3:4, col], obB[0:1, :])
        nc.sync.dma_start(Ud[7:8, col], obB[33:34, :])

    # ---------------- point sets (interleaved schedule) ----------------
    Ud_i = dram.tile([8, NI], F16, tag="Ud_i")
    Ud_l = dram.tile([5, NB], F16, tag="Ud_l")
    Ud_r = dram.tile([5, NB], F16, tag="Ud_r")
    Ud_0 = dram.tile([2, NB], F16, tag="Ud_0")
    bnd_stage = {"l": Ud_l, "r": Ud_r}
    jobs = []
    small = [("l", 0), ("l", 1), ("r", 0), ("r", 1), ("0", 0), ("0", 1)]
    for k in range(NBI):
        jobs.append(("i", k))
        if k % 3 == 2 and small:
            jobs.append(small.pop(0))
    jobs += small
    if KDBG in (2, 3):
        jobs = []
    for set_, k in jobs:
        if set_ == "i":
            emit_block(XiA_sb, XiB_sb, k, 5, Ud_i)
        elif set_ == "l":
            emit_block(XlA_sb, XlB_sb, k, 3, Ud_l)
        elif set_ == "r":
            emit_block(XrA_sb, XrB_sb, k, 3, Ud_r)
        else:
            emit_block(X0A_sb, X0B_sb, k, 1, Ud_0)

    # ---------------- output accumulator ----------------
    out_sb = res.tile([128, OUT_COLS], F32, tag="out_sb")
    if KDBG in (1, 3):
        nc.vector.memset(out_sb[:], 0.0)
        nc.sync.dma_start(io["out"][:], out_sb[:])
        return

    def rt(tag, w=PI):
        return res.tile([128, w], F32, tag=tag, name=tag)

    def tt(out, a, b_, op=OP.mult):
        nc.vector.tensor_tensor(as_ap(out), as_ap(a), as_ap(b_), op)
        return out

    def stt(out, in0, scal, in1, op0=OP.mult, op1=OP.add):
        nc.vector.scalar_tensor_tensor(as_ap(out), as_ap(in0), scal,
                                       as_ap(in1), op0, op1)
        return out

    def sumsq(src, colidx, scale=1.0, w=PI):
        dump = res.tile([128, w], F32, tag="dump", name="dump")
        nc.vector.tensor_tensor(dump[:], as_ap(src), as_ap(src), OP.mult)
        nc.vector.tensor_scalar(
            dump[:], dump[:], float(scale), 0.0, OP.mult, OP.add,
            accum_out=out_sb[:, colidx : colidx + 1])

    # ---------------- boundary residuals ----------------
    for si, (setname, xp, rhs) in enumerate((("l", xl_pm, 0.0), ("r", xr_pm, 1.0))):
        Ub = res.tile([128, 5, PB], F16, tag="Ub", name="Ub")
        nc.sync.dma_start(Ub[:], bnd_stage[setname][:].rearrange(
            "c (p i) -> p c i", p=128))
        ub_ = res.tile([128, PB], F32, tag="bu", name="bu")
        nc.vector.tensor_scalar(ub_[:], Ub[:, BR_U, :], b3u[:], None, OP.add)
        cb = res.tile([128, PB], F32, tag="bc", name="bc")
        nc.vector.tensor_scalar(cb[:], Ub[:, BR_C, :], b3c[:], None, OP.add)
        uxb, cxb, uxxb = Ub[:, BR_UX, :], Ub[:, BR_CX, :], Ub[:, BR_UXX, :]
        bx2 = res.tile([128, PB], F32, tag="bx2", name="bx2")
        nc.scalar.activation(bx2[:], xp[:], AF.Square)
        rx = res.tile([128, PB], F32, tag="rx", name="rx")
        nc.vector.reciprocal(rx[:], xp[:])
        t1 = res.tile([128, PB], F32, tag="t1", name="t1")
        tt(t1, ub_, rx)
        t2 = res.tile([128, PB], F32, tag="t2", name="t2")
        nc.vector.tensor_scalar(t2[:], t1[:], LAM3, None, OP.mult)
        t3 = res.tile([128, PB], F32, tag="t3", name="t3")
        nc.vector.scalar_tensor_tensor(t3[:], uxb, LAM2, t2[:], OP.mult, OP.add)
        t4 = res.tile([128, PB], F32, tag="t4", name="t4")
        nc.vector.scalar_tensor_tensor(t4[:], cb[:], -ALFA / 3.0, t3[:],
                                       OP.mult, OP.add)
        sumsq(t4, 2 + 2 * si, scale=LAM1 * LAM1, w=PB)
        m1b = res.tile([128, PB], F32, tag="m1b", name="m1b")
        nc.vector.tensor_tensor(m1b[:], bx2[:], uxxb, OP.mult)
        m2b = res.tile([128, PB], F32, tag="m2b", name="m2b")
        nc.vector.tensor_tensor(m2b[:], xp[:], uxb, OP.mult)
        m3b = res.tile([128, PB], F32, tag="m3b", name="m3b")
        tt(m3b, m1b, m2b, OP.add)
        m4b = res.tile([128, PB], F32, tag="m4b", name="m4b")
        tt(m4b, m3b, ub_, OP.subtract)
        Cb = res.tile([128, PB], F32, tag="Cb", name="Cb")
        nc.vector.tensor_tensor(Cb[:], bx2[:], cxb, OP.mult)
        m5b = res.tile([128, PB], F32, tag="m5b", name="m5b")
        stt(m5b, Cb, -ALFA, m4b)
        m6b = res.tile([128, PB], F32, tag="m6b", name="m6b")
        tt(m6b, cb, m5b)
        fd = res.tile([128, PB], F32, tag="fd", name="fd")
        stt(fd, m6b, -THETA, Cb)
        if rhs != 0.0:
            fd2 = res.tile([128, PB], F32, tag="fd2", name="fd2")
            tt(fd2, fd, bx2, OP.subtract)
            fd = fd2
        sumsq(fd, 3 + 2 * si, w=PB)

    # ---------------- init residuals ----------------
    U0 = res.tile([128, 2, PB], F16, tag="U0")
    nc.sync.dma_start(U0[:], Ud_0[:].rearrange("c (p i) -> p c i", p=128))
    iu = res.tile([128, PB], F32, tag="iu")
    nc.vector.tensor_scalar(iu[:], U0[:, 0, :], b3u[:], None, OP.add)
    ic = res.tile([128, PB], F32, tag="ic")
    nc.vector.tensor_scalar(ic[:], U0[:, 1, :], b3c[:], None, OP.add)
    for oi, val in enumerate((iu, ic)):
        for j in range(2):
            d = res.tile([128, PB], F32, tag="d0", name="d0")
            nc.vector.tensor_tensor(d[:], val[:], Yi0[:, j, :], OP.subtract)
            dw = res.tile([128, PB], F32, tag="dw", name="dw")
            nc.vector.tensor_tensor(dw[:], d[:], Wi0[:, j, :], OP.mult)
            dump = res.tile([128, PB], F32, tag="dump0", name="dump0")
            nc.vector.tensor_tensor(dump[:], d[:], dw[:], OP.mult)
            nc.vector.tensor_scalar(
                dump[:], dump[:], 1.0, 0.0, OP.mult, OP.add,
                accum_out=out_sb[:, 6 + 2 * oi + j : 7 + 2 * oi + j])

    # ---------------- interior residuals ----------------
    Ui = res.tile([128, 8, PI], F16, tag="Ui")
    nc.sync.dma_start(Ui[:], Ud_i[:].rearrange("c (p i) -> p c i", p=128))
    if KDBG == 5:
        nc.vector.memset(out_sb[:], 0.0)
        nc.vector.tensor_copy(out_sb[:, 0:8], Ui[:, :, 0])
        nc.sync.dma_start(io["out"][:], out_sb[:])
        return
    ub_t = rt("ub_t")
    nc.vector.tensor_scalar(ub_t[:], Ui[:, IR_U, :], b3u[:], None, OP.add)
    cb_t = rt("cb_t")
    nc.vector.tensor_scalar(cb_t[:], Ui[:, IR_C, :], b3c[:], None, OP.add)
    x = x_pm
    x2 = rt("x2")
    nc.scalar.activation(x2[:], x[:], AF.Square)
    x3 = tt(rt("x3"), x2, x)
    A = tt(rt("A"), x2[:], Ui[:, IR_UXX, :])          # x2*uxx
    Bt = tt(rt("Bt"), x[:], Ui[:, IR_UX, :])          # x*ux
    C = tt(rt("C"), x2[:], Ui[:, IR_CX, :])           # x2*cx
    D = tt(rt("D"), x3[:], Ui[:, IR_CXX, :])          # x3*cxx
    E = tt(rt("E"), x3[:], Ui[:, IR_UXXX, :])         # x3*uxxx
    F = tt(rt("F"), x3[:], Ui[:, IR_CT, :])           # x3*ct
    j1 = tt(rt("j1"), A, Bt, OP.add)
    j2 = tt(rt("j2"), j1, ub_t, OP.subtract)     # A+B-u
    fs = stt(rt("fs"), C, -K1, j2)               # -K1*C + j2
    sumsq(fs, 0)
    in2 = stt(rt("in2"), C, -ALFA, j2)           # inner2
    i1 = stt(rt("i1"), A, 2.0, Bt, OP.mult, OP.subtract)   # 2A - B
    i2 = tt(rt("i2"), i1, ub_t, OP.add)
    i3 = stt(rt("i3"), C, -ALFA, E)
    i4 = tt(rt("i4"), i2, i3, OP.add)
    i5 = stt(rt("i5"), D, -ALFA, i4)             # inner1
    k1 = tt(rt("k1"), cb_t, i5)
    k2 = tt(rt("k2"), x, Ui[:, IR_CX, :])        # x*cx
    k3 = tt(rt("k3"), k2, in2)
    k4 = stt(rt("k4"), k1, THETA, F)
    k5 = stt(rt("k5"), k3, THETA, k4)
    k7 = tt(rt("k7"), D, C, OP.add)
    r_ = tt(rt("r_"), k5, k7, OP.subtract)
    sumsq(r_, 1)

    nc.sync.dma_start(io["out"][:], out_sb[:])


def build_nc():
    nc = bacc.Bacc("TRN2", target_bir_lowering=False, debug=False, num_devices=M)
    io = {}

    def dp(name, shape, is_out=False, dt=F32):
        h = nc.declare_dram_parameter(name, list(shape), dt, isOutput=is_out)
        io[name] = h.ap()

    dp("XiA", (2, NI), dt=F16); dp("XiB", (2, NI), dt=F16)
    dp("XlA", (2, NB), dt=F16); dp("XlB", (2, NB), dt=F16)
    dp("XrA", (2, NB), dt=F16); dp("XrB", (2, NB), dt=F16)
    dp("X0A", (2, NB), dt=F16); dp("X0B", (2, NB), dt=F16)
    dp("x_pm", (128, PI)); dp("xl_pm", (128, PB)); dp("xr_pm", (128, PB))
    dp("W0h", (2, H), dt=F16); dp("b0n", (H, 1))
    dp("W1", (H, H), dt=F16); dp("b1", (H, 1))
    dp("W1x", (H, H), dt=F16); dp("W1xx", (H, H), dt=F16)
    dp("W1xxx", (H, H), dt=F16); dp("W1t", (H, H), dt=F16)
    dp("W2", (H, H), dt=F16); dp("b2", (H, 1))
    dp("W3s", (H, 10), dt=F16); dp("b3u", (128, 1)); dp("b3c", (128, 1))
    dp("bqh", (128, 1))
    dp("Wi0", (128, 2, PB)); dp("Yi0", (128, 2, PB))
    dp("out", (128, OUT_COLS), is_out=True)

    with tile.TileContext(nc) as tc:
        with ExitStack() as ctx:
            emit(nc, tc, ctx, io)
    nc.compile()
    return nc


def host_prep(inputs):
    """Fold normalization into layer-0 weights and build the 8 per-core maps."""
    f4, f2 = np.float32, np.float16
    g = {k: np.asarray(v) for k, v in inputs.items()}
    Xint = g["Xint"].astype(f4)
    lb = Xint.min(axis=0).astype(np.float64)
    ub = Xint.max(axis=0).astype(np.float64)
    a = 2.0 / (ub - lb)
    W0 = g["W0"].astype(np.float64)
    W0n = a[:, None] * W0
    beta = -2.0 * lb / (ub - lb) - 1.0
    b0n = beta @ W0 + g["b0"].astype(np.float64)
    W0h = W0n.astype(f2)
    w0x = W0h[0].astype(np.float64)   # match device: fp16 weight actually used

    # diag-folded layer-2 weights (channel scalings s1=1, s2=-1/2, s3=1/12
    # are folded here and un-done per-channel in W3s)
    w0t = W0h[1].astype(np.float64)
    W1f = g["W1"].astype(np.float64)
    W3f = g["W3"].astype(np.float64)
    W3sc = np.concatenate([W3f, W3f, -2.0 * W3f, 12.0 * W3f, W3f], axis=1)
    com = {
        "W0h": W0h,
        "b0n": b0n.astype(f4).reshape(H, 1),
        "W1": g["W1"].astype(f2), "b1": g["b1"].astype(f4).reshape(H, 1),
        "W1x": (w0x[:, None] * W1f).astype(f2),
        "W1xx": (w0x[:, None] ** 2 * W1f).astype(f2),
        "W1xxx": (w0x[:, None] ** 3 * W1f).astype(f2),
        "W1t": (w0t[:, None] * W1f).astype(f2),
        "W2": g["W2"].astype(f2), "b2": g["b2"].astype(f4).reshape(H, 1),
        "W3s": W3sc.astype(f2),
        "b3u": np.full((128, 1), g["b3"][0], f4),
        "b3c": np.full((128, 1), g["b3"][1], f4),
        "bqh": np.full((128, 1), -1.0 / 6.0, f4),
    }

    def split16(X):  # [n,2] fp32 -> (hi, lo) fp16 transposed [2,n]
        XT = np.ascontiguousarray(X.T.astype(f4))
        A = XT.astype(f2)
        Bx = (XT - A.astype(f4)).astype(f2)
        return A, Bx

    in_maps = []
    for c in range(M):
        Xi = Xint[c * NI : (c + 1) * NI]
        Xl = g["Xbnd_l"][c * NB : (c + 1) * NB].astype(f4)
        Xr = g["Xbnd_r"][c * NB : (c + 1) * NB].astype(f4)
        X0 = g["Xinit"][c * NB : (c + 1) * NB].astype(f4)
        Wi = g["Winit"][c * NB : (c + 1) * NB].astype(f4)
        Yi = g["Yinit"][c * NB : (c + 1) * NB].astype(f4)
        m = dict(com)
        m["XiA"], m["XiB"] = split16(Xi)
        m["XlA"], m["XlB"] = split16(Xl)
        m["XrA"], m["XrB"] = split16(Xr)
        m["X0A"], m["X0B"] = split16(X0)
        m["x_pm"] = np.ascontiguousarray(Xi[:, 0].reshape(128, PI))
        m["xl_pm"] = np.ascontiguousarray(Xl[:, 0].reshape(128, PB))
        m["xr_pm"] = np.ascontiguousarray(Xr[:, 0].reshape(128, PB))
        m["Wi0"] = np.ascontiguousarray(Wi.reshape(128, PB, 2).transpose(0, 2, 1))
        m["Yi0"] = np.ascontiguousarray(Yi.reshape(128, PB, 2).transpose(0, 2, 1))
        in_maps.append(m)
    return in_maps


def combine(results):
    s = np.zeros(OUT_COLS, np.float64)
    for r in results:
        s += r["out"].astype(np.float64).sum(axis=0)
    int_loss = (s[0] + s[1]) / N_INT
    bnd_loss = (s[2] + s[3]) / N_BND + (s[4] + s[5]) / N_BND
    init_loss = (s[6] + s[7] + s[8] + s[9]) / (2 * N_BND)
    return np.float32(int_loss + bnd_loss + init_loss)


_CACHE = {}


def _get_nc():
    if "nc" not in _CACHE:
        _CACHE["nc"] = build_nc()
    return _CACHE["nc"]


def kernel(**inputs):
    in_maps = host_prep(inputs)
    nc = _get_nc()
    res = run_bass_kernel_spmd(nc, in_maps, core_ids=list(range(M)))
    return combine(res.results)
